# revision 19
# baseline (speedup 1.0000x reference)
"""Trainium2 Bass kernel for nn_EquivariantProteinGNN (GATv2-style message passing).

Strategy (8 NeuronCores, SPMD):
  - Nodes padded to 20480 and split into 8 contiguous shards of 2560 (20 blocks
    of 128). Edges assigned to the device owning their dst node, sorted by dst,
    and packed into fixed-size per-block runs (CPB chunks of 128 edge slots,
    dummy slots excluded via host-precomputed one-hot matrices).
  - Edge encoder (stage C) runs in transposed space with 512-edge slabs:
    RBF built by a broadcast matmul + Square/Exp activations, eb1 folded via a
    ones-row in rbfT, LayerNorm stats accumulated for ALL blocks into one PSUM
    tile via selector matmuls, then a tiny batched stats pass and a per-block
    normalize pass (raw e2 round-trips DRAM in bf16).
  - Per layer: xl for all blocks -> AllGather (overlapped with xr compute),
    then per-block edge processing: segment softmax and message scatter via
    one-hot matmuls accumulating in PSUM; one-hots come from the host.
  - Pooling: per-graph sums via one-hot matmul, maxes via masked transposed
    reduces; tiny AllGather of partials; head MLP replicated.
"""

import math
import ml_dtypes
import numpy as np

import concourse.bass as bass
import concourse.bacc as bacc
import concourse.mybir as mybir
import concourse.tile as tile
from concourse.bass_utils import run_bass_kernel_spmd
from concourse.masks import make_identity
from concourse.library_config import mlp as mlp_lib

P = 128
D = 384
H, C = 12, 32
NUM_RBF = 100
RBF_MIN, RBF_MAX = 0.0, 30.0
NEG_BIG = -1.0e30
SLAB = 512

f32 = mybir.dt.float32
bf16 = mybir.dt.bfloat16
i32 = mybir.dt.int32
i16 = mybir.dt.int16
AF = mybir.ActivationFunctionType
OP = mybir.AluOpType

HW_ACTS = True

TRACE = False
LAST_RESULTS = None


# --------------------------------------------------------------------------
# host-side preprocessing
# --------------------------------------------------------------------------

def prep_host(inputs, n_dev=8, G=32):
    x = np.asarray(inputs["x"], np.float32)
    pos = np.asarray(inputs["pos"], np.float32)
    edge_index = np.asarray(inputs["edge_index"], np.int64)
    batch = np.asarray(inputs["batch"], np.int64)

    N = x.shape[0]
    E = edge_index.shape[1]
    L = np.asarray(inputs["Wl"]).shape[0]

    PD = int(math.ceil(N / (n_dev * P))) * P          # nodes per device (padded)
    N_pad = PD * n_dev
    NBLK = PD // P

    src = edge_index[0].astype(np.int64)
    dst = edge_index[1].astype(np.int64)

    # edges per 128-node block
    blk = dst // P
    cnt = np.bincount(blk, minlength=N_pad // P)
    CPB = int(math.ceil(cnt.max() / P))
    EPB = CPB * P

    # slot edges: per global block, a run of EPB slots
    order = np.argsort(dst, kind="stable")
    src_s, dst_s = src[order], dst[order]
    blk_s = dst_s // P
    start = np.zeros(len(cnt), np.int64)
    start[1:] = np.cumsum(cnt)[:-1]
    within = np.arange(E) - start[blk_s]
    slot = blk_s * EPB + within                       # global slot id

    n_slots = (N_pad // P) * EPB
    g_src = np.zeros(n_slots, np.int64)
    g_dstrel = np.full(n_slots, -1, np.int64)
    g_dist = np.zeros(n_slots, np.float32)
    g_src[slot] = src_s
    g_dstrel[slot] = dst_s - blk_s * P
    g_dist[slot] = np.linalg.norm(pos[src_s] - pos[dst_s], axis=-1)

    qq = np.arange(P, dtype=np.int64)

    devs = []
    SPD = NBLK * EPB                                  # slots per device
    for d in range(n_dev):
        sl = slice(d * SPD, (d + 1) * SPD)
        gsr = g_src[sl].astype(np.int16).reshape(NBLK, EPB)
        gidx = np.tile(gsr.reshape(NBLK, EPB // 16, 16).transpose(0, 2, 1), (1, 8, 1)).copy()
        dr = g_dstrel[sl].reshape(NBLK, CPB, P)       # [b, c, p]
        # scatter one-hot: ohs[b, p, c, q] = (dr[b, c, p] == q)
        ohs = (dr[..., None] == qq).astype(ml_dtypes.bfloat16).transpose(0, 2, 1, 3).copy()
        # gather one-hot: ohg[b, p, c, q] = (dr[b, c, q] == p)
        ohg = (dr[..., None] == qq).astype(ml_dtypes.bfloat16).transpose(0, 3, 1, 2).copy()
        distT = g_dist[sl].reshape(NBLK, 1, EPB).astype(ml_dtypes.bfloat16)

        # node features, transposed for the embedding matmul
        xdev = np.zeros((PD, x.shape[1]), np.float32)
        lo, hi = d * PD, min((d + 1) * PD, N)
        if hi > lo:
            xdev[: hi - lo] = x[lo:hi]
        xT = np.ascontiguousarray(xdev.T).astype(ml_dtypes.bfloat16)  # (20, PD)

        # pooling helpers
        bdev = np.full(PD, -1, np.int64)
        if hi > lo:
            bdev[: hi - lo] = batch[lo:hi]
        oh = np.zeros((PD, G), np.float32)
        real = bdev >= 0
        oh[np.arange(PD)[real], bdev[real]] = 1.0
        oh = oh.reshape(NBLK, P, G)

        devs.append(dict(gidx=gidx, ohs=ohs, ohg=ohg, distT=distT,
                         xT=xT, oh=oh, bdev=bdev))

    # pooling masks: per block, up to MAXG distinct graphs
    MAXG = 1
    for dv in devs:
        bdev = dv["bdev"]
        for b in range(NBLK):
            u = np.unique(bdev[b * P:(b + 1) * P])
            MAXG = max(MAXG, len(u[u >= 0]))
    for dv in devs:
        bdev = dv.pop("bdev")
        maskG = np.full((NBLK, P, MAXG), NEG_BIG, np.float32)
        cmb = np.full((G, MAXG * NBLK), NEG_BIG, np.float32)
        for b in range(NBLK):
            bb = bdev[b * P:(b + 1) * P]
            u = np.unique(bb)
            u = u[u >= 0]
            for mi, g in enumerate(u):
                maskG[b, :, mi] = np.where(bb == g, 0.0, NEG_BIG)
                cmb[g, MAXG * b + mi] = 0.0
        dv["maskAB"] = maskG
        dv["cmb"] = cmb.reshape(G, 1, MAXG * NBLK)

    # replicated parameter pack
    def bc(v):                                        # [128, n] broadcast
        v = np.asarray(v, np.float32).reshape(1, -1)
        return np.ascontiguousarray(np.broadcast_to(v, (P, v.shape[1])))

    def row(v):
        return np.asarray(v, np.float32).reshape(1, -1)

    def b16(v):
        return np.asarray(v, np.float32).astype(ml_dtypes.bfloat16)

    bn_scale = (np.asarray(inputs["bn_g"], np.float32)
                / np.sqrt(np.asarray(inputs["bn_v"], np.float32) + 1e-5))
    bn_shift = (np.asarray(inputs["bn_b"], np.float32)
                + (np.asarray(inputs["cb"], np.float32)
                   - np.asarray(inputs["bn_m"], np.float32)) * bn_scale)

    centers = np.linspace(RBF_MIN, RBF_MAX, NUM_RBF).astype(np.float32)
    spacing = (RBF_MAX - RBF_MIN) / (NUM_RBF - 1)
    gamma = 1.0 / (spacing ** 2 + 1e-8)

    att = np.asarray(inputs["att"], np.float32).reshape(L, 1, D)
    att_b = np.ascontiguousarray(np.broadcast_to(att, (L, P, D)))
    bnsc_b = np.ascontiguousarray(np.broadcast_to(bn_scale.reshape(L, 1, D), (L, P, D)))
    bnsh_b = np.ascontiguousarray(np.broadcast_to(bn_shift.reshape(L, 1, D), (L, P, D)))

    # eW1 with eb1 folded as an extra contraction row
    eW1s = np.vstack([np.asarray(inputs["eW1"], np.float32),
                      np.asarray(inputs["eb1"], np.float32).reshape(1, D)])

    rep = dict(
        emb_W=b16(inputs["emb_W"]),
        emb_b=b16(row(inputs["emb_b"])),
        emb_g_b=bc(inputs["emb_g"]), emb_beta_b=bc(inputs["emb_beta"]),
        eW1s=b16(eW1s),
        eW2=b16(inputs["eW2"]),
        eb2=b16(row(inputs["eb2"])),
        e_g_col=np.asarray(inputs["e_g"], np.float32).reshape(-1, P).T.copy(),
        e_beta_col=np.asarray(inputs["e_beta"], np.float32).reshape(-1, P).T.copy(),
        neg_centers=-centers.reshape(NUM_RBF, 1),
        iota2n=np.ascontiguousarray(np.broadcast_to(
            np.arange(2 * NBLK, dtype=np.float32), (P, 2 * NBLK))),
        Wl=b16(inputs["Wl"]), bl=b16(np.asarray(inputs["bl"]).reshape(L, 1, D)),
        Wr=b16(inputs["Wr"]), br=b16(np.asarray(inputs["br"]).reshape(L, 1, D)),
        We=b16(inputs["We"]),
        att_b=b16(att_b), bnsc_b=b16(bnsc_b), bnsh_b=b16(bnsh_b),
        pW=np.asarray(inputs["pW"], np.float32), pb=row(inputs["pb"]),
        hW1=np.asarray(inputs["hW1"], np.float32), hb1=row(inputs["hb1"]),
        hW2=np.asarray(inputs["hW2"], np.float32), hb2=row(inputs["hb2"]),
        hW3=np.pad(np.asarray(inputs["hW3"], np.float32), ((0, 64), (0, 0))).reshape(2, P).T.copy(),
        hb3=row(inputs["hb3"]),
    )

    meta = dict(n_dev=n_dev, N=N, E=E, G=G, L=L, PD=PD, N_pad=N_pad,
                NBLK=NBLK, CPB=CPB, EPB=EPB, gamma=gamma,
                x_in=x.shape[1], MAXG=MAXG)
    return meta, rep, devs


# --------------------------------------------------------------------------
# device program
# --------------------------------------------------------------------------

def build_program(meta):
    n_dev = meta["n_dev"]
    L, G = meta["L"], meta["G"]
    PD, N_pad = meta["PD"], meta["N_pad"]
    NBLK, CPB, EPB = meta["NBLK"], meta["CPB"], meta["EPB"]
    MAXG = meta["MAXG"]
    gamma = meta["gamma"]
    XIN = meta["x_in"]
    KD = D // P                                        # 3 feature k-chunks
    slabs = [(s, min(SLAB, EPB - s)) for s in range(0, EPB, SLAB)]

    nc = bacc.Bacc(None, target_bir_lowering=False, debug=False)

    # ---- I/O ----
    def inp(name, shape, dtype=f32):
        return nc.dram_tensor(name, list(shape), dtype, kind="ExternalInput")

    gidx_d = inp("gidx", (NBLK, P, EPB // 16), i16)
    ohs_d = inp("ohs", (NBLK, P, CPB, P), bf16)
    ohg_d = inp("ohg", (NBLK, P, CPB, P), bf16)
    distT_d = inp("distT", (NBLK, 1, EPB), bf16)
    xT_d = inp("xT", (XIN, PD), bf16)
    oh_d = inp("oh", (NBLK, P, G))
    maskAB_d = inp("maskAB", (NBLK, P, MAXG))
    cmb_d = inp("cmb", (G, 1, MAXG * NBLK))

    emb_W_d = inp("emb_W", (XIN, D), bf16)
    emb_b_d = inp("emb_b", (1, D), bf16)
    emb_g_b_d = inp("emb_g_b", (P, D))
    emb_beta_b_d = inp("emb_beta_b", (P, D))
    eW1s_d = inp("eW1s", (NUM_RBF + 1, D), bf16)
    eW2_d = inp("eW2", (D, D), bf16)
    eb2_d = inp("eb2", (1, D), bf16)
    e_g_col_d = inp("e_g_col", (P, KD))
    e_beta_col_d = inp("e_beta_col", (P, KD))
    neg_centers_d = inp("neg_centers", (NUM_RBF, 1))
    iota2n_d = inp("iota2n", (P, 2 * NBLK))
    Wl_d = inp("Wl", (L, D, D), bf16)
    bl_d = inp("bl", (L, 1, D), bf16)
    Wr_d = inp("Wr", (L, D, D), bf16)
    br_d = inp("br", (L, 1, D), bf16)
    We_d = inp("We", (L, D, D), bf16)
    att_b_d = inp("att_b", (L, P, D), bf16)
    bnsc_b_d = inp("bnsc_b", (L, P, D), bf16)
    bnsh_b_d = inp("bnsh_b", (L, P, D), bf16)
    pW_d = inp("pW", (2 * D, D))
    pb_d = inp("pb", (1, D))
    hW1_d = inp("hW1", (D, D))
    hb1_d = inp("hb1", (1, D))
    hW2_d = inp("hW2", (D, D // 2))
    hb2_d = inp("hb2", (1, D // 2))
    hW3_d = inp("hW3", (P, 2))
    hb3_d = inp("hb3", (1, 1))

    out_d = nc.dram_tensor("out", [G], f32, kind="ExternalOutput")

    # internal DRAM
    e2raw_d = nc.dram_tensor("e2raw", [NBLK, KD, P, EPB], bf16)
    encT_d = nc.dram_tensor("encT", [NBLK, KD, P, EPB], bf16)
    stats_d = nc.dram_tensor("stats", [2 * NBLK, EPB], f32)
    xl_shard_d = nc.dram_tensor("xl_shard", [PD, D], bf16)
    shared_as = "Shared" if n_dev > 4 else "Local"
    xl_full_d = nc.dram_tensor("xl_full", [N_pad, D], bf16, addr_space=shared_as)
    pool_part_d = nc.dram_tensor("pool_part", [2 * D + 1, G], f32)
    pool_all_d = nc.dram_tensor("pool_all", [n_dev * (2 * D + 1), G], f32, addr_space=shared_as)

    rg = [list(range(n_dev))]

    with tile.TileContext(nc) as tc:
        with (
            tc.tile_pool(name="consts", bufs=1) as consts,
            tc.tile_pool(name="hpool", bufs=1) as hpool,
        ):
            nc.gpsimd.load_library(mlp_lib)
            ident = consts.tile([P, P], f32, tag="ident")
            make_identity(nc, ident)
            ones_row = consts.tile([1, P], f32, tag="ones_row")
            nc.vector.memset(ones_row[:], 1.0)
            ones_col = consts.tile([P, 1], f32, tag="ones_col")
            nc.vector.memset(ones_col[:], 1.0)
            ones_row_b = consts.tile([1, P], bf16, tag="ones_row_b")
            nc.vector.memset(ones_row_b[:], 1.0)
            ones_sq_b = consts.tile([P, SLAB], bf16, tag="ones_sq_b")
            nc.vector.memset(ones_sq_b[:], 1.0)
            ones_1r100 = consts.tile([1, NUM_RBF], bf16, tag="ones_1r100")
            nc.vector.memset(ones_1r100[:], 1.0)
            eps_col = consts.tile([P, 1], f32, tag="eps_col")
            nc.vector.memset(eps_col[:], 1e-5)

            silu_n = [0]

            def emit_silu(pool, out_ap, in_ap, shape):
                # silu(x) = x / (1 + exp(-x)); single-table (exp) formulation
                silu_n[0] += 1
                sn = silu_n[0]
                ex = pool.tile(shape, f32, tag="silu_ex", name=f"silu_ex{sn}")
                nc.scalar.activation(ex[:], in_ap, AF.Exp, scale=-1.0)
                nc.vector.tensor_scalar(out=ex[:], in0=ex[:], scalar1=1.0,
                                        scalar2=None, op0=OP.add)
                rcp = pool.tile(shape, f32, tag="silu_rc", name=f"silu_rc{sn}")
                nc.vector.reciprocal_approx_fast(rcp[:], ex[:])
                nc.vector.tensor_tensor(out=out_ap, in0=in_ap, in1=rcp[:], op=OP.mult)

            h_sb = [hpool.tile([P, D], f32, tag=f"h{b}", name=f"h{b}")
                    for b in range(NBLK)]

            # =========================================================
            # Stage B: node embedding  h0 = silu(LN(x @ emb_W + emb_b))
            # =========================================================
            with (
                tc.tile_pool(name="embsb", bufs=2) as embsb,
                tc.tile_pool(name="embc", bufs=1) as embc,
                tc.tile_pool(name="embxc", bufs=1) as embxc,
                tc.tile_pool(name="embps", bufs=2, space="PSUM") as embps,
            ):
                xT_sb = embc.tile([XIN, PD], bf16, tag="xT")
                nc.sync.dma_start(xT_sb[:], xT_d[:, :])
                embW_sb = embc.tile([XIN, D], bf16, tag="embW")
                nc.sync.dma_start(embW_sb[:], emb_W_d[:, :])
                embb_sb = embc.tile([1, D], bf16, tag="embb")
                nc.sync.dma_start(embb_sb[:], emb_b_d[:, :])
                emb_g_sb = embc.tile([P, D], f32, tag="embg")
                nc.sync.dma_start(emb_g_sb[:], emb_g_b_d[:, :])
                emb_beta_sb = embc.tile([P, D], f32, tag="embbeta")
                nc.sync.dma_start(emb_beta_sb[:], emb_beta_b_d[:, :])
                var_all = embc.tile([P, NBLK], f32, tag="var_all")
                rstd_all = embc.tile([P, NBLK], f32, tag="rstd_all")
                xc_all = [embxc.tile([P, D], f32, tag=f"xc{b}", name=f"xc{b}")
                          for b in range(NBLK)]

                # pass 1: matmul + center + accumulate var; Ln batched after
                for b in range(NBLK):
                    ps = embps.tile([P, D], f32, tag="ps")
                    nc.tensor.matmul(ps[:], xT_sb[:, b * P:(b + 1) * P], embW_sb[:],
                                     start=True, stop=False)
                    nc.tensor.matmul(ps[:], ones_row_b[:, :P], embb_sb[:],
                                     start=False, stop=True)
                    mu = embsb.tile([P, 1], f32, tag="mu")
                    nc.vector.tensor_reduce(out=mu[:], in_=ps[:],
                                            axis=mybir.AxisListType.X, op=OP.add)
                    nc.vector.tensor_scalar(out=mu[:], in0=mu[:], scalar1=1.0 / D,
                                            scalar2=None, op0=OP.mult)
                    xc = xc_all[b]
                    nc.vector.tensor_scalar(out=xc[:], in0=ps[:], scalar1=mu[:, :1],
                                            scalar2=None, op0=OP.subtract)
                    sq = embsb.tile([P, D], f32, tag="sq")
                    nc.scalar.activation(sq[:], xc[:], AF.Square,
                                         accum_out=var_all[:, b:b + 1])
                # one Ln/Exp pair for all blocks (avoids act-table thrash)
                nc.scalar.activation(rstd_all[:], var_all[:], AF.Ln, scale=1.0 / D,
                                     bias=eps_col[:, :1])
                nc.scalar.activation(rstd_all[:], rstd_all[:], AF.Exp, scale=-0.5)
                for b in range(NBLK):
                    xc = xc_all[b]
                    nc.vector.tensor_scalar(out=xc[:], in0=xc[:],
                                            scalar1=rstd_all[:, b:b + 1],
                                            scalar2=None, op0=OP.mult)
                    nc.vector.tensor_tensor(out=xc[:], in0=xc[:], in1=emb_g_sb[:], op=OP.mult)
                    nc.vector.tensor_tensor(out=xc[:], in0=xc[:], in1=emb_beta_sb[:], op=OP.add)
                    emit_silu(embsb, h_sb[b][:], xc[:], [P, D])

            # =========================================================
            # Stage C: edge encoder -> encT
            # =========================================================
            with tc.tile_pool(name="encstat", bufs=1) as encstat:
                eg_sb = encstat.tile([P, KD], f32, tag="eg")
                nc.sync.dma_start(eg_sb[:], e_g_col_d[:, :])
                ebeta_sb = encstat.tile([P, KD], f32, tag="ebeta")
                nc.sync.dma_start(ebeta_sb[:], e_beta_col_d[:, :])
                rstd16 = encstat.tile([NBLK, EPB], bf16, tag="rstd16")
                nmu16 = encstat.tile([NBLK, EPB], bf16, tag="nmu16")

                # ---- pass 1: raw e2 (pre-LN) in slabs + stats accumulation ----
                with (
                    tc.tile_pool(name="encw", bufs=1) as encw,
                    tc.tile_pool(name="encsb", bufs=3) as encsb,
                    tc.tile_pool(name="encrb", bufs=1) as encrb,
                    tc.tile_pool(name="pcrbf", bufs=1, space="PSUM") as pcrbf,
                    tc.tile_pool(name="pce1", bufs=2, space="PSUM") as pce1,
                    tc.tile_pool(name="pce2", bufs=2, space="PSUM") as pce2,
                    tc.tile_pool(name="pcstat", bufs=1, space="PSUM") as pcstat,
                ):
                    eW1s_sb = encw.tile([NUM_RBF + 1, D], bf16, tag="eW1s")
                    nc.sync.dma_start(eW1s_sb[:], eW1s_d[:, :])
                    eW2_sb = [encw.tile([P, D], bf16, tag=f"eW2_{k}", name=f"eW2_{k}")
                              for k in range(KD)]
                    for k in range(KD):
                        nc.sync.dma_start(eW2_sb[k][:], eW2_d[k * P:(k + 1) * P, :])
                    eb2_sb = encw.tile([1, D], bf16, tag="eb2")
                    nc.sync.dma_start(eb2_sb[:], eb2_d[:, :])
                    negc_sb = encw.tile([NUM_RBF, 1], f32, tag="negc")
                    nc.sync.dma_start(negc_sb[:], neg_centers_d[:, :])
                    iota2n_sb = encw.tile([P, 2 * NBLK], f32, tag="iota2n")
                    nc.sync.dma_start(iota2n_sb[:], iota2n_d[:, :])

                    # persistent rbfT tiles (one per slab index) with a ones row
                    rbfT_sb = [encrb.tile([NUM_RBF + 1, w], bf16, tag=f"rbfT{si}",
                                          name=f"rbfT{si}")
                               for si, (s0, w) in enumerate(slabs)]
                    for t, (s0, w) in zip(rbfT_sb, slabs):
                        # engines cannot address partition base 100; DMA can
                        nc.sync.dma_start(t[NUM_RBF:NUM_RBF + 1, :],
                                          ones_sq_b[:1, :w])

                    # stats PSUM: rows 0..N-1 = sum(e2), rows N..2N-1 = sum(e2^2)
                    stat_ps = [pcstat.tile([2 * NBLK, w], f32, tag=f"stat{si}",
                                           name=f"stat{si}")
                               for si, (s0, w) in enumerate(slabs)]

                    for b in range(NBLK):
                        dist_row = encsb.tile([1, EPB], bf16, tag="dist_row")
                        nc.sync.dma_start(dist_row[:], distT_d[b])
                        sel1 = encsb.tile([P, 2 * NBLK], bf16, tag="sel1")
                        nc.vector.tensor_scalar(out=sel1[:], in0=iota2n_sb[:],
                                                scalar1=float(b), scalar2=None,
                                                op0=OP.is_equal)
                        sel2 = encsb.tile([P, 2 * NBLK], bf16, tag="sel2")
                        nc.vector.tensor_scalar(out=sel2[:], in0=iota2n_sb[:],
                                                scalar1=float(NBLK + b), scalar2=None,
                                                op0=OP.is_equal)
                        for si, (s0, w) in enumerate(slabs):
                            # RBF: broadcast dist row, then (d-c)^2, exp(-g u)
                            rbf_ps = pcrbf.tile([NUM_RBF, SLAB], f32, tag="rbf_ps")
                            nc.tensor.matmul(rbf_ps[:, :w], ones_1r100[:, :],
                                             dist_row[:, s0:s0 + w],
                                             start=True, stop=True)
                            u_sb = encsb.tile([NUM_RBF, SLAB], f32, tag="u_sb")
                            nc.scalar.activation(u_sb[:, :w], rbf_ps[:, :w], AF.Square,
                                                 bias=negc_sb[:, :1])
                            nc.scalar.activation(rbfT_sb[si][:NUM_RBF, :w], u_sb[:, :w],
                                                 AF.Exp, scale=-float(gamma))
                            # e1 = silu(rbfT.T @ [eW1; eb1])  (transposed, per k)
                            e1s = []
                            for k in range(KD):
                                e1_ps = pce1.tile([P, SLAB], f32, tag="e1_ps")
                                nc.tensor.matmul(e1_ps[:, :w],
                                                 eW1s_sb[:, k * P:(k + 1) * P],
                                                 rbfT_sb[si][:, :w],
                                                 start=True, stop=True)
                                ex = encsb.tile([P, SLAB], f32, tag="ex")
                                nc.scalar.activation(ex[:, :w], e1_ps[:, :w], AF.Exp,
                                                     scale=-1.0)
                                nc.vector.tensor_scalar(out=ex[:, :w], in0=ex[:, :w],
                                                        scalar1=1.0, scalar2=None,
                                                        op0=OP.add)
                                rcp = encsb.tile([P, SLAB], f32, tag="rcp")
                                nc.vector.reciprocal_approx_fast(rcp[:, :w], ex[:, :w])
                                t = encsb.tile([P, SLAB], bf16, tag=f"e1s{k}",
                                               name=f"e1s{k}")
                                nc.vector.tensor_tensor(out=t[:, :w], in0=e1_ps[:, :w],
                                                        in1=rcp[:, :w], op=OP.mult)
                                e1s.append(t)
                            # e2 = e1 @ eW2 + eb2 (transposed, per out chunk m)
                            for m in range(KD):
                                e2_ps = pce2.tile([P, SLAB], f32, tag="e2_ps")
                                for k in range(KD):
                                    nc.tensor.matmul(e2_ps[:, :w],
                                                     eW2_sb[k][:, m * P:(m + 1) * P],
                                                     e1s[k][:, :w],
                                                     start=(k == 0), stop=False)
                                nc.tensor.matmul(e2_ps[:, :w],
                                                 eb2_sb[:, m * P:(m + 1) * P],
                                                 ones_sq_b[:1, :w],
                                                 start=False, stop=True)
                                e2s = encsb.tile([P, SLAB], bf16, tag="e2s")
                                nc.scalar.copy(e2s[:, :w], e2_ps[:, :w])
                                sq = encsb.tile([P, SLAB], bf16, tag="sqs")
                                nc.vector.tensor_tensor(out=sq[:, :w], in0=e2s[:, :w],
                                                        in1=e2s[:, :w], op=OP.mult)
                                acc0 = b == 0 and m == 0
                                nc.tensor.matmul(stat_ps[si][:, :w], sel1[:], e2s[:, :w],
                                                 start=acc0, stop=False)
                                last = b == NBLK - 1 and m == KD - 1
                                nc.tensor.matmul(stat_ps[si][:, :w], sel2[:], sq[:, :w],
                                                 start=False, stop=last)
                                nc.sync.dma_start(e2raw_d[b, m, :, s0:s0 + w],
                                                  e2s[:, :w])

                    # ---- pass 2: batched LayerNorm stats for all blocks ----
                    # round-trip through DRAM so mu/r2 halves can be re-read at
                    # partition base 0 (engines need aligned operand bases)
                    statsA = encw.tile([2 * NBLK, EPB], f32, tag="statsA")
                    for si, (s0, w) in enumerate(slabs):
                        nc.vector.tensor_copy(statsA[:, s0:s0 + w], stat_ps[si][:, :w])
                    nc.sync.dma_start(stats_d[:, :], statsA[:])
                    muD = encw.tile([NBLK, EPB], f32, tag="muD")
                    nc.sync.dma_start(muD[:], stats_d[:NBLK, :])
                    r2D = encw.tile([NBLK, EPB], f32, tag="r2D")
                    nc.sync.dma_start(r2D[:], stats_d[NBLK:, :])
                    nc.vector.tensor_scalar(out=muD[:], in0=muD[:], scalar1=1.0 / D,
                                            scalar2=None, op0=OP.mult)
                    mu2 = encw.tile([NBLK, EPB], f32, tag="mu2")
                    nc.scalar.activation(mu2[:], muD[:], AF.Square)
                    var = encw.tile([NBLK, EPB], f32, tag="var")
                    nc.vector.scalar_tensor_tensor(out=var[:], in0=r2D[:],
                                                   scalar=1.0 / D, in1=mu2[:],
                                                   op0=OP.mult, op1=OP.subtract)
                    nc.vector.tensor_scalar(out=var[:], in0=var[:], scalar1=0.0,
                                            scalar2=None, op0=OP.max)
                    nc.scalar.activation(var[:], var[:], AF.Ln, bias=eps_col[:NBLK, :1])
                    nc.scalar.activation(var[:], var[:], AF.Exp, scale=-0.5)  # rstd
                    nmu = encw.tile([NBLK, EPB], f32, tag="nmu")
                    nc.vector.scalar_tensor_tensor(out=nmu[:], in0=muD[:], scalar=-1.0,
                                                   in1=var[:], op0=OP.mult, op1=OP.mult)
                    nc.vector.tensor_copy(rstd16[:], var[:])
                    nc.vector.tensor_copy(nmu16[:], nmu[:])

                # ---- pass 3: enc = e2*outer(g,rstd) + outer(g,nmu) + beta ----
                with (
                    tc.tile_pool(name="enc3", bufs=2) as enc3,
                    tc.tile_pool(name="pc3a", bufs=3, space="PSUM") as pc3a,
                    tc.tile_pool(name="pc3b", bufs=3, space="PSUM") as pc3b,
                ):
                    for b in range(NBLK):
                        e2t = [enc3.tile([P, EPB], bf16, tag=f"e2t{k}",
                                         name=f"e2t{k}") for k in range(KD)]
                        wout = [enc3.tile([P, EPB], bf16, tag=f"wout{k}",
                                          name=f"wout{k}") for k in range(KD)]
                        for k in range(KD):
                            nc.sync.dma_start(e2t[k][:], e2raw_d[b, k])
                        # per-block rows at partition base 0 (SBUF->SBUF DMA)
                        rstd_row = enc3.tile([1, EPB], bf16, tag="rstd_row")
                        nc.sync.dma_start(rstd_row[:], rstd16[b:b + 1, :])
                        nmu_row = enc3.tile([1, EPB], bf16, tag="nmu_row")
                        nc.sync.dma_start(nmu_row[:], nmu16[b:b + 1, :])
                        for s0, w in slabs:
                            # rstd/nmu broadcast across partitions, shared by
                            # all 3 feature chunks; g/beta applied per chunk
                            # as per-partition scalars
                            pA = pc3a.tile([P, SLAB], f32, tag="pA")
                            nc.tensor.matmul(pA[:, :w], ones_row_b[:, :],
                                             rstd_row[:, s0:s0 + w],
                                             start=True, stop=True)
                            pB = pc3b.tile([P, SLAB], f32, tag="pB")
                            nc.tensor.matmul(pB[:, :w], ones_row_b[:, :],
                                             nmu_row[:, s0:s0 + w],
                                             start=True, stop=True)
                            for m in range(KD):
                                t1 = enc3.tile([P, SLAB], f32, tag="t1")
                                nc.vector.tensor_tensor(out=t1[:, :w],
                                                        in0=e2t[m][:, s0:s0 + w],
                                                        in1=pA[:, :w], op=OP.mult)
                                nc.vector.tensor_tensor(out=t1[:, :w],
                                                        in0=t1[:, :w],
                                                        in1=pB[:, :w], op=OP.add)
                                nc.vector.tensor_scalar(
                                    out=wout[m][:, s0:s0 + w], in0=t1[:, :w],
                                    scalar1=eg_sb[:, m:m + 1],
                                    scalar2=ebeta_sb[:, m:m + 1],
                                    op0=OP.mult, op1=OP.add)
                        for k in range(KD):
                            nc.sync.dma_start(encT_d[b, k], wout[k][:])

            # =========================================================
            # Main layers
            # =========================================================
            with (
                tc.tile_pool(name="xrpool", bufs=1) as xrpool,
                tc.tile_pool(name="htpool", bufs=1) as htpool,
                tc.tile_pool(name="lw", bufs=2) as lw,
                tc.tile_pool(name="lep", bufs=1) as lep,
                tc.tile_pool(name="lsb", bufs=2) as lsb,
                tc.tile_pool(name="gat", bufs=2) as gat,
                tc.tile_pool(name="eetp", bufs=2) as eetp,
                tc.tile_pool(name="lps", bufs=3, space="PSUM") as lps,
                tc.tile_pool(name="lpt", bufs=2, space="PSUM") as lpt,
                tc.tile_pool(name="lpo", bufs=2, space="PSUM") as lpo,
            ):
                xr_sb = [xrpool.tile([P, D], bf16, tag=f"xr{b}", name=f"xr{b}")
                         for b in range(NBLK)]
                hT_sb = [[htpool.tile([P, P], bf16, tag=f"hT{b}_{k}",
                                      name=f"hT{b}_{k}") for k in range(KD)]
                         for b in range(NBLK)]
                for layer in range(L):
                    # ---- layer weights ----
                    Wl_sb = [lw.tile([P, D], bf16, tag=f"Wl{k}", name=f"Wl{k}")
                             for k in range(KD)]
                    Wr_sb = [lw.tile([P, D], bf16, tag=f"Wr{k}", name=f"Wr{k}")
                             for k in range(KD)]
                    We_sb = [lw.tile([P, D], bf16, tag=f"We{k}", name=f"We{k}")
                             for k in range(KD)]
                    for k in range(KD):
                        nc.sync.dma_start(Wl_sb[k][:], Wl_d[layer, k * P:(k + 1) * P, :])
                        nc.sync.dma_start(Wr_sb[k][:], Wr_d[layer, k * P:(k + 1) * P, :])
                        nc.sync.dma_start(We_sb[k][:], We_d[layer, k * P:(k + 1) * P, :])
                    bl_sb = lw.tile([1, D], bf16, tag="bl")
                    nc.sync.dma_start(bl_sb[:], bl_d[layer])
                    br_sb = lw.tile([1, D], bf16, tag="br")
                    nc.sync.dma_start(br_sb[:], br_d[layer])
                    attb_sb = lw.tile([P, D], bf16, tag="attb")
                    nc.sync.dma_start(attb_sb[:], att_b_d[layer])
                    bnsc_sb = lw.tile([P, D], bf16, tag="bnsc")
                    nc.sync.dma_start(bnsc_sb[:], bnsc_b_d[layer])
                    bnsh_sb = lw.tile([P, D], bf16, tag="bnsh")
                    nc.sync.dma_start(bnsh_sb[:], bnsh_b_d[layer])

                    # ---- stage D part 1: hT + xl for every block ----
                    for b in range(NBLK):
                        for k in range(KD):
                            pt = lpt.tile([P, P], f32, tag="pt")
                            nc.tensor.transpose(pt[:], h_sb[b][:, k * P:(k + 1) * P], ident[:])
                            nc.vector.tensor_copy(hT_sb[b][k][:], pt[:])
                        pxl = lps.tile([P, D], f32, tag="ps")
                        for k in range(KD):
                            nc.tensor.matmul(pxl[:], hT_sb[b][k][:], Wl_sb[k][:],
                                             start=(k == 0), stop=False)
                        nc.tensor.matmul(pxl[:], ones_row_b[:, :P], bl_sb[:],
                                         start=False, stop=True)
                        xl_t = lsb.tile([P, D], bf16, tag="xl_t")
                        nc.vector.tensor_copy(xl_t[:], pxl[:])
                        nc.sync.dma_start(xl_shard_d[b * P:(b + 1) * P, :], xl_t[:])

                    # ---- AllGather xl (xr compute overlaps with it) ----
                    nc.gpsimd.collective_compute(
                        "AllGather", OP.bypass, replica_groups=rg,
                        ins=[xl_shard_d[:, :]], outs=[xl_full_d[:, :]],
                    )

                    # ---- stage D part 2: xr for every block ----
                    for b in range(NBLK):
                        pxr = lps.tile([P, D], f32, tag="ps")
                        for k in range(KD):
                            nc.tensor.matmul(pxr[:], hT_sb[b][k][:], Wr_sb[k][:],
                                             start=(k == 0), stop=False)
                        nc.tensor.matmul(pxr[:], ones_row_b[:, :P], br_sb[:],
                                         start=False, stop=True)
                        nc.vector.tensor_copy(xr_sb[b][:], pxr[:])

                    # ---- stage E: edge message passing ----
                    eo_all = lep.tile([P, NBLK, D + H], bf16, tag="eo_all")
                    for b in range(NBLK):
                        gix = gat.tile([P, EPB // 16], i16, tag="gix")
                        nc.sync.dma_start(gix[:], gidx_d[b])
                        ohs_t = gat.tile([P, CPB, P], bf16, tag="ohs_t")
                        nc.sync.dma_start(ohs_t[:], ohs_d[b])
                        ohg_t = gat.tile([P, CPB, P], bf16, tag="ohg_t")
                        nc.sync.dma_start(ohg_t[:], ohg_d[b])
                        eet = [eetp.tile([P, EPB], bf16, tag=f"eet{k}", name=f"eet{k}")
                               for k in range(KD)]
                        for k in range(KD):
                            nc.sync.dma_start(eet[k][:], encT_d[b, k])
                        xsg = eetp.tile([P, CPB, D], bf16, tag="xsg")
                        nc.gpsimd.dma_gather(xsg[:], xl_full_d[:, :], gix[:], EPB, EPB, D,
                                             single_packet=False)
                        psum_o = lpo.tile([P, D + H], f32, tag="po")
                        for c in range(CPB):
                            xsrc = xsg[:, c]
                            psum_s = lps.tile([P, D], f32, tag="ps")
                            for k in range(KD):
                                nc.tensor.matmul(psum_s[:], eet[k][:, c * P:(c + 1) * P],
                                                 We_sb[k][:], start=(k == 0), stop=False)
                            nc.tensor.matmul(psum_s[:], ohg_t[:, c, :], xr_sb[b][:],
                                             start=False, stop=True)
                            s_sb = lsb.tile([P, D], bf16, tag="s_sb")
                            nc.vector.tensor_tensor(out=s_sb[:], in0=psum_s[:],
                                                    in1=xsrc, op=OP.add)
                            m_sb = lsb.tile([P, D], bf16, tag="m_sb")
                            if HW_ACTS:
                                nc.scalar.activation(m_sb[:], s_sb[:], AF.Prelu, alpha=0.2)
                            else:
                                nc.scalar.activation(m_sb[:], s_sb[:], AF.Relu)
                            t_sb = lsb.tile([P, D], bf16, tag="t_sb")
                            nc.gpsimd.tensor_tensor(out=t_sb[:], in0=m_sb[:], in1=attb_sb[:],
                                                    op=OP.mult)
                            lg = lsb.tile([P, H], f32, tag="lg")
                            nc.vector.tensor_reduce(
                                out=lg[:], in_=t_sb[:].rearrange("p (h c) -> p h c", h=H),
                                axis=mybir.AxisListType.X, op=OP.add)
                            z_sb = lsb.tile([P, D + H], bf16, tag="z_sb")
                            nc.scalar.activation(z_sb[:, D:], lg[:], AF.Exp)
                            el_b = z_sb[:, D:].rearrange("p (h o) -> p h o", o=1).to_broadcast([P, H, C])
                            nc.vector.tensor_tensor(
                                out=z_sb[:, :D].rearrange("p (h c) -> p h c", h=H),
                                in0=xsrc.rearrange("p (h c) -> p h c", h=H),
                                in1=el_b, op=OP.mult)
                            nc.tensor.matmul(psum_o[:], ohs_t[:, c, :], z_sb[:],
                                             start=(c == 0), stop=(c == CPB - 1))
                        # stash numerators+denominators; epilogue is batched
                        nc.scalar.copy(eo_all[:, b, :], psum_o[:])

                    # ---- batched epilogue for all blocks ----
                    den_f = lep.tile([P, NBLK, H], f32, tag="den_f")
                    nc.vector.tensor_scalar(out=den_f[:], in0=eo_all[:, :, D:],
                                            scalar1=1e-16, scalar2=None, op0=OP.add)
                    rec_f = lep.tile([P, NBLK, H], f32, tag="rec_f")
                    nc.vector.reciprocal_approx_fast(rec_f[:], den_f[:])
                    rec_b16 = lep.tile([P, NBLK, H], bf16, tag="rec_b16")
                    nc.vector.tensor_copy(rec_b16[:], rec_f[:])
                    num_v = eo_all[:, :, :D].rearrange("p n (h c) -> p n h c", h=H)
                    rec_v = rec_b16[:].rearrange("p n (h o) -> p n h o", o=1) \
                        .to_broadcast([P, NBLK, H, C])
                    nc.vector.tensor_tensor(out=num_v, in0=num_v, in1=rec_v,
                                            op=OP.mult)
                    o1_v = eo_all[:, :, :D]
                    bnsc_v = bnsc_sb[:].rearrange("p (o d) -> p o d", o=1) \
                        .to_broadcast([P, NBLK, D])
                    bnsh_v = bnsh_sb[:].rearrange("p (o d) -> p o d", o=1) \
                        .to_broadcast([P, NBLK, D])
                    nc.vector.tensor_tensor(out=o1_v, in0=o1_v, in1=bnsc_v, op=OP.mult)
                    nc.vector.tensor_tensor(out=o1_v, in0=o1_v, in1=bnsh_v, op=OP.add)
                    ex_f = lep.tile([P, NBLK, D], f32, tag="ex_f")
                    nc.scalar.activation(ex_f[:], o1_v, AF.Exp, scale=-1.0)
                    nc.vector.tensor_scalar(out=ex_f[:], in0=ex_f[:], scalar1=1.0,
                                            scalar2=None, op0=OP.add)
                    nc.vector.reciprocal_approx_fast(ex_f[:], ex_f[:])
                    nc.vector.tensor_tensor(out=o1_v, in0=o1_v, in1=ex_f[:],
                                            op=OP.mult)
                    for b in range(NBLK):
                        nc.vector.tensor_tensor(out=h_sb[b][:], in0=h_sb[b][:],
                                                in1=eo_all[:, b, :D], op=OP.add)

            # =========================================================
            # Stage F: pooling + head
            # =========================================================
            with (
                tc.tile_pool(name="fsb", bufs=3) as fsb,
                tc.tile_pool(name="fkeep", bufs=1) as fkeep,
                tc.tile_pool(name="fps", bufs=2, space="PSUM") as fps,
                tc.tile_pool(name="fsum", bufs=1, space="PSUM") as fsum,
            ):
                psum_sum = fsum.tile([G, D], f32, tag="psum_sum")
                psum_cnt = fsum.tile([1, G], f32, tag="psum_cnt")
                bm = [fkeep.tile([P, MAXG * NBLK], f32, tag=f"bm{k}", name=f"bm{k}")
                      for k in range(KD)]
                for b in range(NBLK):
                    ohb = fsb.tile([P, G], f32, tag="ohb")
                    nc.sync.dma_start(ohb[:], oh_d[b])
                    mab = fsb.tile([P, MAXG], f32, tag="mab")
                    nc.sync.dma_start(mab[:], maskAB_d[b])
                    nc.tensor.matmul(psum_sum[:], ohb[:], h_sb[b][:],
                                     start=(b == 0), stop=(b == NBLK - 1))
                    nc.tensor.matmul(psum_cnt[:], ones_col[:, :1], ohb[:],
                                     start=(b == 0), stop=(b == NBLK - 1))
                    for half in range(MAXG):
                        mh = fsb.tile([P, D], f32, tag="mh")
                        nc.vector.tensor_scalar(out=mh[:], in0=h_sb[b][:],
                                                scalar1=mab[:, half:half + 1], scalar2=None,
                                                op0=OP.add)
                        for k in range(KD):
                            pt = fps.tile([P, P], f32, tag="pt")
                            nc.tensor.transpose(pt[:], mh[:, k * P:(k + 1) * P], ident[:])
                            mt = fsb.tile([P, P], f32, tag="mt")
                            nc.vector.tensor_copy(mt[:], pt[:])
                            nc.vector.tensor_reduce(
                                out=bm[k][:, MAXG * b + half:MAXG * b + half + 1],
                                in_=mt[:], axis=mybir.AxisListType.X, op=OP.max)
                # combine per-graph maxes
                gmaxT = [fkeep.tile([P, G], f32, tag=f"gmaxT{k}", name=f"gmaxT{k}")
                         for k in range(KD)]
                for g in range(G):
                    cr = fsb.tile([1, MAXG * NBLK], f32, tag="cr")
                    nc.sync.dma_start(cr[:], cmb_d[g])
                    pc = fps.tile([P, MAXG * NBLK], f32, tag="pt")
                    nc.tensor.matmul(pc[:], ones_row[:, :P], cr[:], start=True, stop=True)
                    for k in range(KD):
                        mm = fsb.tile([P, MAXG * NBLK], f32, tag="mm")
                        nc.vector.tensor_tensor(out=mm[:], in0=bm[k][:], in1=pc[:], op=OP.add)
                        nc.vector.tensor_reduce(out=gmaxT[k][:, g:g + 1], in_=mm[:],
                                                axis=mybir.AxisListType.X, op=OP.max)
                # partial sums to DRAM
                sum_sb = fsb.tile([G, D], f32, tag="sum_sb")
                nc.vector.tensor_copy(sum_sb[:], psum_sum[:])
                for k in range(KD):
                    pt = fps.tile([P, G], f32, tag="pt")
                    nc.tensor.transpose(pt[:, :G], sum_sb[:, k * P:(k + 1) * P], ident[:G, :G])
                    st = fsb.tile([P, G], f32, tag="st")
                    nc.vector.tensor_copy(st[:], pt[:, :G])
                    nc.sync.dma_start(pool_part_d[k * P:(k + 1) * P, :], st[:])
                    nc.sync.dma_start(pool_part_d[D + k * P:D + (k + 1) * P, :], gmaxT[k][:])
                cntT = fsb.tile([1, G], f32, tag="cntT")
                nc.vector.tensor_copy(cntT[:], psum_cnt[:])
                nc.sync.dma_start(pool_part_d[2 * D:2 * D + 1, :], cntT[:])

                # ---- tiny AllGather of partials ----
                nc.gpsimd.collective_compute(
                    "AllGather", OP.bypass, replica_groups=rg,
                    ins=[pool_part_d[:, :]], outs=[pool_all_d[:, :]],
                )

                # ---- combine + head (replicated on all devices) ----
                n_dev_ = n_dev
                STRIDE = 2 * D + 1
                meanT = [fkeep.tile([P, G], f32, tag=f"meanT{k}", name=f"meanT{k}")
                         for k in range(KD)]
                maxT = [fkeep.tile([P, G], f32, tag=f"maxT{k}", name=f"maxT{k}")
                        for k in range(KD)]
                cnt_tot = fkeep.tile([1, G], f32, tag="cnt_tot")
                for dv in range(n_dev_):
                    base = dv * STRIDE
                    for k in range(KD):
                        ts = fsb.tile([P, G], f32, tag="ts")
                        nc.sync.dma_start(ts[:], pool_all_d[base + k * P:base + (k + 1) * P, :])
                        tm = fsb.tile([P, G], f32, tag="tm")
                        nc.sync.dma_start(tm[:], pool_all_d[base + D + k * P:base + D + (k + 1) * P, :])
                        if dv == 0:
                            nc.vector.tensor_copy(meanT[k][:], ts[:])
                            nc.vector.tensor_copy(maxT[k][:], tm[:])
                        else:
                            nc.vector.tensor_tensor(out=meanT[k][:], in0=meanT[k][:],
                                                    in1=ts[:], op=OP.add)
                            nc.vector.tensor_tensor(out=maxT[k][:], in0=maxT[k][:],
                                                    in1=tm[:], op=OP.max)
                    tc_ = fsb.tile([1, G], f32, tag="tc_")
                    nc.sync.dma_start(tc_[:], pool_all_d[base + 2 * D:base + 2 * D + 1, :])
                    if dv == 0:
                        nc.vector.tensor_copy(cnt_tot[:], tc_[:])
                    else:
                        nc.vector.tensor_tensor(out=cnt_tot[:], in0=cnt_tot[:], in1=tc_[:],
                                                op=OP.add)
                nc.vector.tensor_scalar(out=cnt_tot[:], in0=cnt_tot[:], scalar1=1.0,
                                        scalar2=None, op0=OP.max)
                inv_cnt = fkeep.tile([1, G], f32, tag="inv_cnt")
                nc.vector.reciprocal(inv_cnt[:], cnt_tot[:])
                pic = fps.tile([P, G], f32, tag="pt")
                nc.tensor.matmul(pic[:], ones_row[:, :P], inv_cnt[:], start=True, stop=True)
                for k in range(KD):
                    nc.vector.tensor_tensor(out=meanT[k][:], in0=meanT[k][:], in1=pic[:],
                                            op=OP.mult)
                hgT = meanT + maxT          # 6 k-tiles of [128, G] = hg transposed

                # head weights
                pW_sb = [fkeep.tile([P, D], f32, tag=f"pW{k}", name=f"pW{k}")
                         for k in range(2 * KD)]
                for k in range(2 * KD):
                    nc.sync.dma_start(pW_sb[k][:], pW_d[k * P:(k + 1) * P, :])
                pb_sb = fkeep.tile([1, D], f32, tag="pb")
                nc.sync.dma_start(pb_sb[:], pb_d[:, :])
                hW1_sb = [fkeep.tile([P, D], f32, tag=f"hW1_{k}", name=f"hW1_{k}")
                          for k in range(KD)]
                for k in range(KD):
                    nc.sync.dma_start(hW1_sb[k][:], hW1_d[k * P:(k + 1) * P, :])
                hb1_sb = fkeep.tile([1, D], f32, tag="hb1")
                nc.sync.dma_start(hb1_sb[:], hb1_d[:, :])
                hW2_sb = [fkeep.tile([P, D // 2], f32, tag=f"hW2_{k}", name=f"hW2_{k}")
                          for k in range(KD)]
                for k in range(KD):
                    nc.sync.dma_start(hW2_sb[k][:], hW2_d[k * P:(k + 1) * P, :])
                hb2_sb = fkeep.tile([1, D // 2], f32, tag="hb2")
                nc.sync.dma_start(hb2_sb[:], hb2_d[:, :])
                hW3_sb = fkeep.tile([P, 2], f32, tag="hW3")
                nc.sync.dma_start(hW3_sb[:], hW3_d[:, :].rearrange("(k p) o -> p (k o)", p=P))
                hb3_sb = fkeep.tile([1, 1], f32, tag="hb3")
                nc.sync.dma_start(hb3_sb[:], hb3_d[:, :])

                def mlp_layer(in_tiles, W_tiles, b_row, out_feats, lid, act=True):
                    outs = []
                    n_out_tiles = (out_feats + P - 1) // P
                    for m in range(n_out_tiles):
                        mw = min(P, out_feats - m * P)
                        pm = fps.tile([P, G], f32, tag="ph", name=f"ph{lid}_{m}")
                        for k, (it, wt) in enumerate(zip(in_tiles, W_tiles)):
                            nc.tensor.matmul(pm[:mw, :], wt[:, m * P:m * P + mw], it[:],
                                             start=(k == 0), stop=False)
                        nc.tensor.matmul(pm[:mw, :], b_row[:, m * P:m * P + mw],
                                         ones_row[:, :G], start=False, stop=True)
                        ot = fkeep.tile([P, G], f32, tag=f"ot{lid}_{m}", name=f"ot{lid}_{m}")
                        if mw < P:
                            nc.vector.memset(ot[mw:, :], 0.0)
                        if act:
                            emit_silu(fsb, ot[:mw, :], pm[:mw, :], [mw, G])
                        else:
                            nc.vector.tensor_copy(ot[:mw, :], pm[:mw, :])
                        outs.append(ot)
                    return outs

                h1 = mlp_layer(hgT, pW_sb, pb_sb, D, 1)
                h2 = mlp_layer(h1, hW1_sb, hb1_sb, D, 2)
                h3 = mlp_layer(h2, hW2_sb, hb2_sb, D // 2, 3)
                pf = fps.tile([1, G], f32, tag="pf")
                nc.tensor.matmul(pf[:], hW3_sb[:, 0:1], h3[0][:], start=True, stop=False)
                nc.tensor.matmul(pf[:], hW3_sb[:, 1:2], h3[1][:], start=False, stop=False)
                nc.tensor.matmul(pf[:], hb3_sb[:, :1], ones_row[:, :G], start=False, stop=True)
                fo = fsb.tile([1, G], f32, tag="fo")
                nc.vector.tensor_copy(fo[:], pf[:])
                nc.sync.dma_start(out_d[:].rearrange("(o g) -> o g", o=1), fo[:])

    nc.compile()
    return nc


# --------------------------------------------------------------------------
# entry point
# --------------------------------------------------------------------------

def kernel(**inputs):
    n_dev = 8
    meta, rep, devs = prep_host(inputs, n_dev)
    nc = build_program(meta)

    in_maps = []
    for d in range(n_dev):
        m = dict(rep)
        m.update(devs[d])
        in_maps.append(m)

    global LAST_RESULTS
    res = run_bass_kernel_spmd(nc, in_maps, core_ids=list(range(n_dev)),
                               trace=TRACE)
    LAST_RESULTS = res
    out = np.asarray(res.results[0]["out"], np.float32)
    return out


# revision 20
# speedup vs baseline: 1.5914x; 1.5914x over previous
"""Trainium2 Bass kernel for nn_EquivariantProteinGNN (GATv2-style message passing).

Strategy (8 NeuronCores, SPMD):
  - Nodes padded to 20480 and split into 8 contiguous shards of 2560 (20 blocks
    of 128). Edges assigned to the device owning their dst node, sorted by dst,
    and packed into fixed-size per-block runs (CPB chunks of 128 edge slots,
    dummy slots excluded via host-precomputed one-hot matrices).
  - Edge encoder (stage C) runs in transposed space with 512-edge slabs:
    RBF built by a broadcast matmul + Square/Exp activations, eb1 folded via a
    ones-row in rbfT, LayerNorm stats accumulated for ALL blocks into one PSUM
    tile via selector matmuls, then a tiny batched stats pass and a per-block
    normalize pass (raw e2 round-trips DRAM in bf16).
  - Per layer: xl for all blocks -> AllGather (overlapped with xr compute),
    then per-block edge processing: segment softmax and message scatter via
    one-hot matmuls accumulating in PSUM; one-hots come from the host.
  - Pooling: per-graph sums via one-hot matmul, maxes via masked transposed
    reduces; tiny AllGather of partials; head MLP replicated.
"""

import math
import ml_dtypes
import numpy as np

import concourse.bass as bass
import concourse.bacc as bacc
import concourse.mybir as mybir
import concourse.tile as tile
from concourse.bass_utils import run_bass_kernel_spmd
from concourse.masks import make_identity
from concourse.library_config import mlp as mlp_lib

P = 128
D = 384
H, C = 12, 32
NUM_RBF = 100
RBF_MIN, RBF_MAX = 0.0, 30.0
NEG_BIG = -1.0e30
SLAB = 512

f32 = mybir.dt.float32
bf16 = mybir.dt.bfloat16
i32 = mybir.dt.int32
i16 = mybir.dt.int16
AF = mybir.ActivationFunctionType
OP = mybir.AluOpType

HW_ACTS = True

TRACE = False
LAST_RESULTS = None


# --------------------------------------------------------------------------
# host-side preprocessing
# --------------------------------------------------------------------------

def prep_host(inputs, n_dev=8, G=32):
    x = np.asarray(inputs["x"], np.float32)
    pos = np.asarray(inputs["pos"], np.float32)
    edge_index = np.asarray(inputs["edge_index"], np.int64)
    batch = np.asarray(inputs["batch"], np.int64)

    N = x.shape[0]
    E = edge_index.shape[1]
    L = np.asarray(inputs["Wl"]).shape[0]

    PD = int(math.ceil(N / (n_dev * P))) * P          # nodes per device (padded)
    N_pad = PD * n_dev
    NBLK = PD // P

    src = edge_index[0].astype(np.int64)
    dst = edge_index[1].astype(np.int64)

    # edges per 128-node block
    blk = dst // P
    cnt = np.bincount(blk, minlength=N_pad // P)
    CPB = int(math.ceil(cnt.max() / P))
    EPB = CPB * P

    # slot edges: per global block, a run of EPB slots
    order = np.argsort(dst, kind="stable")
    src_s, dst_s = src[order], dst[order]
    blk_s = dst_s // P
    start = np.zeros(len(cnt), np.int64)
    start[1:] = np.cumsum(cnt)[:-1]
    within = np.arange(E) - start[blk_s]
    slot = blk_s * EPB + within                       # global slot id

    n_slots = (N_pad // P) * EPB
    g_src = np.zeros(n_slots, np.int64)
    g_dstrel = np.full(n_slots, -1, np.int64)
    g_dist = np.zeros(n_slots, np.float32)
    g_src[slot] = src_s
    g_dstrel[slot] = dst_s - blk_s * P
    g_dist[slot] = np.linalg.norm(pos[src_s] - pos[dst_s], axis=-1)

    qq = np.arange(P, dtype=np.int64)

    devs = []
    SPD = NBLK * EPB                                  # slots per device
    for d in range(n_dev):
        sl = slice(d * SPD, (d + 1) * SPD)
        gsr = g_src[sl].astype(np.int16).reshape(NBLK, EPB)
        gidx = np.tile(gsr.reshape(NBLK, EPB // 16, 16).transpose(0, 2, 1), (1, 8, 1)).copy()
        dr = g_dstrel[sl].reshape(NBLK, CPB, P)       # [b, c, p]
        # scatter one-hot: ohs[b, p, c, q] = (dr[b, c, p] == q)
        ohs = (dr[..., None] == qq).astype(ml_dtypes.bfloat16).transpose(0, 2, 1, 3).copy()
        # gather one-hot: ohg[b, p, c, q] = (dr[b, c, q] == p)
        ohg = (dr[..., None] == qq).astype(ml_dtypes.bfloat16).transpose(0, 3, 1, 2).copy()
        distT = g_dist[sl].reshape(NBLK, 1, EPB).astype(ml_dtypes.bfloat16)

        # node features, transposed for the embedding matmul
        xdev = np.zeros((PD, x.shape[1]), np.float32)
        lo, hi = d * PD, min((d + 1) * PD, N)
        if hi > lo:
            xdev[: hi - lo] = x[lo:hi]
        xT = np.ascontiguousarray(xdev.T).astype(ml_dtypes.bfloat16)  # (20, PD)

        # pooling helpers
        bdev = np.full(PD, -1, np.int64)
        if hi > lo:
            bdev[: hi - lo] = batch[lo:hi]
        oh = np.zeros((PD, G), np.float32)
        real = bdev >= 0
        oh[np.arange(PD)[real], bdev[real]] = 1.0
        oh = oh.reshape(NBLK, P, G)

        devs.append(dict(gidx=gidx, ohs=ohs, ohg=ohg, distT=distT,
                         xT=xT, oh=oh, bdev=bdev))

    # pooling masks: per block, up to MAXG distinct graphs
    MAXG = 1
    for dv in devs:
        bdev = dv["bdev"]
        for b in range(NBLK):
            u = np.unique(bdev[b * P:(b + 1) * P])
            MAXG = max(MAXG, len(u[u >= 0]))
    for dv in devs:
        bdev = dv.pop("bdev")
        maskG = np.full((NBLK, P, MAXG), NEG_BIG, np.float32)
        cmb = np.full((G, MAXG * NBLK), NEG_BIG, np.float32)
        for b in range(NBLK):
            bb = bdev[b * P:(b + 1) * P]
            u = np.unique(bb)
            u = u[u >= 0]
            for mi, g in enumerate(u):
                maskG[b, :, mi] = np.where(bb == g, 0.0, NEG_BIG)
                cmb[g, MAXG * b + mi] = 0.0
        dv["maskAB"] = maskG
        dv["cmb"] = cmb.reshape(G, 1, MAXG * NBLK)

    # replicated parameter pack
    def bc(v):                                        # [128, n] broadcast
        v = np.asarray(v, np.float32).reshape(1, -1)
        return np.ascontiguousarray(np.broadcast_to(v, (P, v.shape[1])))

    def row(v):
        return np.asarray(v, np.float32).reshape(1, -1)

    def b16(v):
        return np.asarray(v, np.float32).astype(ml_dtypes.bfloat16)

    bn_scale = (np.asarray(inputs["bn_g"], np.float32)
                / np.sqrt(np.asarray(inputs["bn_v"], np.float32) + 1e-5))
    bn_shift = (np.asarray(inputs["bn_b"], np.float32)
                + (np.asarray(inputs["cb"], np.float32)
                   - np.asarray(inputs["bn_m"], np.float32)) * bn_scale)

    centers = np.linspace(RBF_MIN, RBF_MAX, NUM_RBF).astype(np.float32)
    spacing = (RBF_MAX - RBF_MIN) / (NUM_RBF - 1)
    gamma = 1.0 / (spacing ** 2 + 1e-8)

    att = np.asarray(inputs["att"], np.float32).reshape(L, 1, D)
    att_b = np.ascontiguousarray(np.broadcast_to(att, (L, P, D)))
    bnsc_b = np.ascontiguousarray(np.broadcast_to(bn_scale.reshape(L, 1, D), (L, P, D)))
    bnsh_b = np.ascontiguousarray(np.broadcast_to(bn_shift.reshape(L, 1, D), (L, P, D)))

    # eW1 with eb1 folded as an extra contraction row
    eW1s = np.vstack([np.asarray(inputs["eW1"], np.float32),
                      np.asarray(inputs["eb1"], np.float32).reshape(1, D)])

    rep = dict(
        emb_W=b16(inputs["emb_W"]),
        emb_b=b16(row(inputs["emb_b"])),
        emb_g_b=bc(inputs["emb_g"]), emb_beta_b=bc(inputs["emb_beta"]),
        eW1s=b16(eW1s),
        eW2=b16(inputs["eW2"]),
        eb2=b16(row(inputs["eb2"])),
        e_g_col=np.asarray(inputs["e_g"], np.float32).reshape(-1, P).T.copy(),
        e_beta_col=np.asarray(inputs["e_beta"], np.float32).reshape(-1, P).T.copy(),
        neg_centers=-centers.reshape(NUM_RBF, 1),
        iota2n=np.ascontiguousarray(np.broadcast_to(
            np.arange(2 * NBLK, dtype=np.float32), (P, 2 * NBLK))),
        Wl=b16(inputs["Wl"]), bl=b16(np.asarray(inputs["bl"]).reshape(L, 1, D)),
        Wr=b16(inputs["Wr"]), br=b16(np.asarray(inputs["br"]).reshape(L, 1, D)),
        We=b16(inputs["We"]),
        att_b=b16(att_b), bnsc_b=b16(bnsc_b), bnsh_b=b16(bnsh_b),
        pW=np.asarray(inputs["pW"], np.float32), pb=row(inputs["pb"]),
        hW1=np.asarray(inputs["hW1"], np.float32), hb1=row(inputs["hb1"]),
        hW2=np.asarray(inputs["hW2"], np.float32), hb2=row(inputs["hb2"]),
        hW3=np.pad(np.asarray(inputs["hW3"], np.float32), ((0, 64), (0, 0))).reshape(2, P).T.copy(),
        hb3=row(inputs["hb3"]),
    )

    meta = dict(n_dev=n_dev, N=N, E=E, G=G, L=L, PD=PD, N_pad=N_pad,
                NBLK=NBLK, CPB=CPB, EPB=EPB, gamma=gamma,
                x_in=x.shape[1], MAXG=MAXG)
    return meta, rep, devs


# --------------------------------------------------------------------------
# device program
# --------------------------------------------------------------------------

def build_program(meta):
    n_dev = meta["n_dev"]
    L, G = meta["L"], meta["G"]
    PD, N_pad = meta["PD"], meta["N_pad"]
    NBLK, CPB, EPB = meta["NBLK"], meta["CPB"], meta["EPB"]
    MAXG = meta["MAXG"]
    gamma = meta["gamma"]
    XIN = meta["x_in"]
    KD = D // P                                        # 3 feature k-chunks
    slabs = [(s, min(SLAB, EPB - s)) for s in range(0, EPB, SLAB)]

    nc = bacc.Bacc(None, target_bir_lowering=False, debug=False)

    # ---- I/O ----
    def inp(name, shape, dtype=f32):
        return nc.dram_tensor(name, list(shape), dtype, kind="ExternalInput")

    gidx_d = inp("gidx", (NBLK, P, EPB // 16), i16)
    ohs_d = inp("ohs", (NBLK, P, CPB, P), bf16)
    ohg_d = inp("ohg", (NBLK, P, CPB, P), bf16)
    distT_d = inp("distT", (NBLK, 1, EPB), bf16)
    xT_d = inp("xT", (XIN, PD), bf16)
    oh_d = inp("oh", (NBLK, P, G))
    maskAB_d = inp("maskAB", (NBLK, P, MAXG))
    cmb_d = inp("cmb", (G, 1, MAXG * NBLK))

    emb_W_d = inp("emb_W", (XIN, D), bf16)
    emb_b_d = inp("emb_b", (1, D), bf16)
    emb_g_b_d = inp("emb_g_b", (P, D))
    emb_beta_b_d = inp("emb_beta_b", (P, D))
    eW1s_d = inp("eW1s", (NUM_RBF + 1, D), bf16)
    eW2_d = inp("eW2", (D, D), bf16)
    eb2_d = inp("eb2", (1, D), bf16)
    e_g_col_d = inp("e_g_col", (P, KD))
    e_beta_col_d = inp("e_beta_col", (P, KD))
    neg_centers_d = inp("neg_centers", (NUM_RBF, 1))
    iota2n_d = inp("iota2n", (P, 2 * NBLK))
    Wl_d = inp("Wl", (L, D, D), bf16)
    bl_d = inp("bl", (L, 1, D), bf16)
    Wr_d = inp("Wr", (L, D, D), bf16)
    br_d = inp("br", (L, 1, D), bf16)
    We_d = inp("We", (L, D, D), bf16)
    att_b_d = inp("att_b", (L, P, D), bf16)
    bnsc_b_d = inp("bnsc_b", (L, P, D), bf16)
    bnsh_b_d = inp("bnsh_b", (L, P, D), bf16)
    pW_d = inp("pW", (2 * D, D))
    pb_d = inp("pb", (1, D))
    hW1_d = inp("hW1", (D, D))
    hb1_d = inp("hb1", (1, D))
    hW2_d = inp("hW2", (D, D // 2))
    hb2_d = inp("hb2", (1, D // 2))
    hW3_d = inp("hW3", (P, 2))
    hb3_d = inp("hb3", (1, 1))

    out_d = nc.dram_tensor("out", [G], f32, kind="ExternalOutput")

    # internal DRAM
    e2raw_d = nc.dram_tensor("e2raw", [NBLK, KD, P, EPB], bf16)
    encT_d = nc.dram_tensor("encT", [NBLK, KD, P, EPB], bf16)
    stats_d = nc.dram_tensor("stats", [2 * NBLK, EPB], f32)
    xl_shard_d = nc.dram_tensor("xl_shard", [PD, D], bf16)
    shared_as = "Shared" if n_dev > 4 else "Local"
    xl_full_d = nc.dram_tensor("xl_full", [N_pad, D], bf16, addr_space=shared_as)
    pool_part_d = nc.dram_tensor("pool_part", [2 * D + 1, G], f32)
    pool_all_d = nc.dram_tensor("pool_all", [n_dev * (2 * D + 1), G], f32, addr_space=shared_as)

    rg = [list(range(n_dev))]

    with tile.TileContext(nc) as tc:
        with (
            tc.tile_pool(name="consts", bufs=1) as consts,
            tc.tile_pool(name="hpool", bufs=1) as hpool,
        ):
            nc.gpsimd.load_library(mlp_lib)
            ident = consts.tile([P, P], f32, tag="ident")
            make_identity(nc, ident)
            ones_row = consts.tile([1, P], f32, tag="ones_row")
            nc.vector.memset(ones_row[:], 1.0)
            ones_col = consts.tile([P, 1], f32, tag="ones_col")
            nc.vector.memset(ones_col[:], 1.0)
            ones_row_b = consts.tile([1, P], bf16, tag="ones_row_b")
            nc.vector.memset(ones_row_b[:], 1.0)
            ones_sq_b = consts.tile([P, SLAB], bf16, tag="ones_sq_b")
            nc.vector.memset(ones_sq_b[:], 1.0)
            ones_1r100 = consts.tile([1, NUM_RBF], bf16, tag="ones_1r100")
            nc.vector.memset(ones_1r100[:], 1.0)
            eps_col = consts.tile([P, 1], f32, tag="eps_col")
            nc.vector.memset(eps_col[:], 1e-5)

            silu_n = [0]

            def emit_silu(pool, out_ap, in_ap, shape):
                # silu(x) = x / (1 + exp(-x)); single-table (exp) formulation
                silu_n[0] += 1
                sn = silu_n[0]
                ex = pool.tile(shape, f32, tag="silu_ex", name=f"silu_ex{sn}")
                nc.scalar.activation(ex[:], in_ap, AF.Exp, scale=-1.0)
                nc.vector.tensor_scalar(out=ex[:], in0=ex[:], scalar1=1.0,
                                        scalar2=None, op0=OP.add)
                rcp = pool.tile(shape, f32, tag="silu_rc", name=f"silu_rc{sn}")
                nc.vector.reciprocal_approx_fast(rcp[:], ex[:])
                nc.vector.tensor_tensor(out=out_ap, in0=in_ap, in1=rcp[:], op=OP.mult)

            h_sb = [hpool.tile([P, D], f32, tag=f"h{b}", name=f"h{b}")
                    for b in range(NBLK)]

            # =========================================================
            # Stage B: node embedding  h0 = silu(LN(x @ emb_W + emb_b))
            # =========================================================
            with (
                tc.tile_pool(name="embsb", bufs=2) as embsb,
                tc.tile_pool(name="embc", bufs=1) as embc,
                tc.tile_pool(name="embxc", bufs=1) as embxc,
                tc.tile_pool(name="embps", bufs=2, space="PSUM") as embps,
            ):
                xT_sb = embc.tile([XIN, PD], bf16, tag="xT")
                nc.sync.dma_start(xT_sb[:], xT_d[:, :])
                embW_sb = embc.tile([XIN, D], bf16, tag="embW")
                nc.sync.dma_start(embW_sb[:], emb_W_d[:, :])
                embb_sb = embc.tile([1, D], bf16, tag="embb")
                nc.sync.dma_start(embb_sb[:], emb_b_d[:, :])
                emb_g_sb = embc.tile([P, D], f32, tag="embg")
                nc.sync.dma_start(emb_g_sb[:], emb_g_b_d[:, :])
                emb_beta_sb = embc.tile([P, D], f32, tag="embbeta")
                nc.sync.dma_start(emb_beta_sb[:], emb_beta_b_d[:, :])
                var_all = embc.tile([P, NBLK], f32, tag="var_all")
                rstd_all = embc.tile([P, NBLK], f32, tag="rstd_all")
                xc_all = [embxc.tile([P, D], f32, tag=f"xc{b}", name=f"xc{b}")
                          for b in range(NBLK)]

                # pass 1: matmul + center + accumulate var; Ln batched after
                for b in range(NBLK):
                    ps = embps.tile([P, D], f32, tag="ps")
                    nc.tensor.matmul(ps[:], xT_sb[:, b * P:(b + 1) * P], embW_sb[:],
                                     start=True, stop=False)
                    nc.tensor.matmul(ps[:], ones_row_b[:, :P], embb_sb[:],
                                     start=False, stop=True)
                    mu = embsb.tile([P, 1], f32, tag="mu")
                    nc.vector.tensor_reduce(out=mu[:], in_=ps[:],
                                            axis=mybir.AxisListType.X, op=OP.add)
                    nc.vector.tensor_scalar(out=mu[:], in0=mu[:], scalar1=1.0 / D,
                                            scalar2=None, op0=OP.mult)
                    xc = xc_all[b]
                    nc.vector.tensor_scalar(out=xc[:], in0=ps[:], scalar1=mu[:, :1],
                                            scalar2=None, op0=OP.subtract)
                    sq = embsb.tile([P, D], f32, tag="sq")
                    nc.scalar.activation(sq[:], xc[:], AF.Square,
                                         accum_out=var_all[:, b:b + 1])
                # one Ln/Exp pair for all blocks (avoids act-table thrash)
                nc.scalar.activation(rstd_all[:], var_all[:], AF.Ln, scale=1.0 / D,
                                     bias=eps_col[:, :1])
                nc.scalar.activation(rstd_all[:], rstd_all[:], AF.Exp, scale=-0.5)
                for b in range(NBLK):
                    xc = xc_all[b]
                    nc.vector.tensor_scalar(out=xc[:], in0=xc[:],
                                            scalar1=rstd_all[:, b:b + 1],
                                            scalar2=None, op0=OP.mult)
                    nc.vector.tensor_tensor(out=xc[:], in0=xc[:], in1=emb_g_sb[:], op=OP.mult)
                    nc.vector.tensor_tensor(out=xc[:], in0=xc[:], in1=emb_beta_sb[:], op=OP.add)
                    emit_silu(embsb, h_sb[b][:], xc[:], [P, D])

            # =========================================================
            # Stage C: edge encoder -> encT
            # =========================================================
            with tc.tile_pool(name="encstat", bufs=1) as encstat:
                eg_sb = encstat.tile([P, KD], f32, tag="eg")
                nc.sync.dma_start(eg_sb[:], e_g_col_d[:, :])
                ebeta_sb = encstat.tile([P, KD], f32, tag="ebeta")
                nc.sync.dma_start(ebeta_sb[:], e_beta_col_d[:, :])
                rstd16 = encstat.tile([NBLK, EPB], bf16, tag="rstd16")
                nmu16 = encstat.tile([NBLK, EPB], bf16, tag="nmu16")

                # ---- pass 1: raw e2 (pre-LN) in slabs + stats accumulation ----
                with (
                    tc.tile_pool(name="encw", bufs=1) as encw,
                    tc.tile_pool(name="encsb", bufs=3) as encsb,
                    tc.tile_pool(name="encrb", bufs=1) as encrb,
                    tc.tile_pool(name="pcrbf", bufs=1, space="PSUM") as pcrbf,
                    tc.tile_pool(name="pce1", bufs=2, space="PSUM") as pce1,
                    tc.tile_pool(name="pce2", bufs=2, space="PSUM") as pce2,
                    tc.tile_pool(name="pcstat", bufs=1, space="PSUM") as pcstat,
                ):
                    eW1s_sb = encw.tile([NUM_RBF + 1, D], bf16, tag="eW1s")
                    nc.sync.dma_start(eW1s_sb[:], eW1s_d[:, :])
                    eW2_sb = [encw.tile([P, D], bf16, tag=f"eW2_{k}", name=f"eW2_{k}")
                              for k in range(KD)]
                    for k in range(KD):
                        nc.sync.dma_start(eW2_sb[k][:], eW2_d[k * P:(k + 1) * P, :])
                    eb2_sb = encw.tile([1, D], bf16, tag="eb2")
                    nc.sync.dma_start(eb2_sb[:], eb2_d[:, :])
                    negc_sb = encw.tile([NUM_RBF, 1], f32, tag="negc")
                    nc.sync.dma_start(negc_sb[:], neg_centers_d[:, :])
                    iota2n_sb = encw.tile([P, 2 * NBLK], f32, tag="iota2n")
                    nc.sync.dma_start(iota2n_sb[:], iota2n_d[:, :])

                    # persistent rbfT tiles (one per slab index) with a ones row
                    rbfT_sb = [encrb.tile([NUM_RBF + 1, w], bf16, tag=f"rbfT{si}",
                                          name=f"rbfT{si}")
                               for si, (s0, w) in enumerate(slabs)]
                    for t, (s0, w) in zip(rbfT_sb, slabs):
                        # engines cannot address partition base 100; DMA can
                        nc.sync.dma_start(t[NUM_RBF:NUM_RBF + 1, :],
                                          ones_sq_b[:1, :w])

                    # stats PSUM: rows 0..N-1 = sum(e2), rows N..2N-1 = sum(e2^2)
                    stat_ps = [pcstat.tile([2 * NBLK, w], f32, tag=f"stat{si}",
                                           name=f"stat{si}")
                               for si, (s0, w) in enumerate(slabs)]

                    for b in range(NBLK):
                        dist_row = encsb.tile([1, EPB], bf16, tag="dist_row")
                        nc.sync.dma_start(dist_row[:], distT_d[b])
                        sel1 = encsb.tile([P, 2 * NBLK], bf16, tag="sel1")
                        nc.vector.tensor_scalar(out=sel1[:], in0=iota2n_sb[:],
                                                scalar1=float(b), scalar2=None,
                                                op0=OP.is_equal)
                        sel2 = encsb.tile([P, 2 * NBLK], bf16, tag="sel2")
                        nc.vector.tensor_scalar(out=sel2[:], in0=iota2n_sb[:],
                                                scalar1=float(NBLK + b), scalar2=None,
                                                op0=OP.is_equal)
                        for si, (s0, w) in enumerate(slabs):
                            # RBF: broadcast dist row, then (d-c)^2, exp(-g u)
                            rbf_ps = pcrbf.tile([NUM_RBF, SLAB], f32, tag="rbf_ps")
                            nc.tensor.matmul(rbf_ps[:, :w], ones_1r100[:, :],
                                             dist_row[:, s0:s0 + w],
                                             start=True, stop=True)
                            u_sb = encsb.tile([NUM_RBF, SLAB], f32, tag="u_sb")
                            nc.scalar.activation(u_sb[:, :w], rbf_ps[:, :w], AF.Square,
                                                 bias=negc_sb[:, :1])
                            nc.scalar.activation(rbfT_sb[si][:NUM_RBF, :w], u_sb[:, :w],
                                                 AF.Exp, scale=-float(gamma))
                            # e1 = silu(rbfT.T @ [eW1; eb1])  (transposed, per k)
                            e1s = []
                            for k in range(KD):
                                e1_ps = pce1.tile([P, SLAB], f32, tag="e1_ps")
                                nc.tensor.matmul(e1_ps[:, :w],
                                                 eW1s_sb[:, k * P:(k + 1) * P],
                                                 rbfT_sb[si][:, :w],
                                                 start=True, stop=True)
                                ex = encsb.tile([P, SLAB], f32, tag="ex")
                                nc.scalar.activation(ex[:, :w], e1_ps[:, :w], AF.Exp,
                                                     scale=-1.0)
                                nc.vector.tensor_scalar(out=ex[:, :w], in0=ex[:, :w],
                                                        scalar1=1.0, scalar2=None,
                                                        op0=OP.add)
                                rcp = encsb.tile([P, SLAB], f32, tag="rcp")
                                nc.vector.reciprocal_approx_fast(rcp[:, :w], ex[:, :w])
                                t = encsb.tile([P, SLAB], bf16, tag=f"e1s{k}",
                                               name=f"e1s{k}")
                                nc.vector.tensor_tensor(out=t[:, :w], in0=e1_ps[:, :w],
                                                        in1=rcp[:, :w], op=OP.mult)
                                e1s.append(t)
                            # e2 = e1 @ eW2 + eb2 (transposed, per out chunk m)
                            for m in range(KD):
                                e2_ps = pce2.tile([P, SLAB], f32, tag="e2_ps")
                                for k in range(KD):
                                    nc.tensor.matmul(e2_ps[:, :w],
                                                     eW2_sb[k][:, m * P:(m + 1) * P],
                                                     e1s[k][:, :w],
                                                     start=(k == 0), stop=False)
                                nc.tensor.matmul(e2_ps[:, :w],
                                                 eb2_sb[:, m * P:(m + 1) * P],
                                                 ones_sq_b[:1, :w],
                                                 start=False, stop=True)
                                e2s = encsb.tile([P, SLAB], bf16, tag="e2s")
                                nc.scalar.copy(e2s[:, :w], e2_ps[:, :w])
                                sq = encsb.tile([P, SLAB], bf16, tag="sqs")
                                nc.vector.tensor_tensor(out=sq[:, :w], in0=e2s[:, :w],
                                                        in1=e2s[:, :w], op=OP.mult)
                                acc0 = b == 0 and m == 0
                                nc.tensor.matmul(stat_ps[si][:, :w], sel1[:], e2s[:, :w],
                                                 start=acc0, stop=False)
                                last = b == NBLK - 1 and m == KD - 1
                                nc.tensor.matmul(stat_ps[si][:, :w], sel2[:], sq[:, :w],
                                                 start=False, stop=last)
                                nc.sync.dma_start(e2raw_d[b, m, :, s0:s0 + w],
                                                  e2s[:, :w])

                    # ---- pass 2: batched LayerNorm stats for all blocks ----
                    # round-trip through DRAM so mu/r2 halves can be re-read at
                    # partition base 0 (engines need aligned operand bases)
                    statsA = encw.tile([2 * NBLK, EPB], f32, tag="statsA")
                    for si, (s0, w) in enumerate(slabs):
                        nc.vector.tensor_copy(statsA[:, s0:s0 + w], stat_ps[si][:, :w])
                    nc.sync.dma_start(stats_d[:, :], statsA[:])
                    muD = encw.tile([NBLK, EPB], f32, tag="muD")
                    nc.sync.dma_start(muD[:], stats_d[:NBLK, :])
                    r2D = encw.tile([NBLK, EPB], f32, tag="r2D")
                    nc.sync.dma_start(r2D[:], stats_d[NBLK:, :])
                    nc.vector.tensor_scalar(out=muD[:], in0=muD[:], scalar1=1.0 / D,
                                            scalar2=None, op0=OP.mult)
                    mu2 = encw.tile([NBLK, EPB], f32, tag="mu2")
                    nc.scalar.activation(mu2[:], muD[:], AF.Square)
                    var = encw.tile([NBLK, EPB], f32, tag="var")
                    nc.vector.scalar_tensor_tensor(out=var[:], in0=r2D[:],
                                                   scalar=1.0 / D, in1=mu2[:],
                                                   op0=OP.mult, op1=OP.subtract)
                    nc.vector.tensor_scalar(out=var[:], in0=var[:], scalar1=0.0,
                                            scalar2=None, op0=OP.max)
                    nc.scalar.activation(var[:], var[:], AF.Ln, bias=eps_col[:NBLK, :1])
                    nc.scalar.activation(var[:], var[:], AF.Exp, scale=-0.5)  # rstd
                    nmu = encw.tile([NBLK, EPB], f32, tag="nmu")
                    nc.vector.scalar_tensor_tensor(out=nmu[:], in0=muD[:], scalar=-1.0,
                                                   in1=var[:], op0=OP.mult, op1=OP.mult)
                    nc.vector.tensor_copy(rstd16[:], var[:])
                    nc.vector.tensor_copy(nmu16[:], nmu[:])

                # ---- pass 3: enc = e2*outer(g,rstd) + outer(g,nmu) + beta ----
                with (
                    tc.tile_pool(name="enc3", bufs=2) as enc3,
                    tc.tile_pool(name="pc3a", bufs=3, space="PSUM") as pc3a,
                    tc.tile_pool(name="pc3b", bufs=3, space="PSUM") as pc3b,
                ):
                    for b in range(NBLK):
                        e2t = [enc3.tile([P, EPB], bf16, tag=f"e2t{k}",
                                         name=f"e2t{k}") for k in range(KD)]
                        wout = [enc3.tile([P, EPB], bf16, tag=f"wout{k}",
                                          name=f"wout{k}") for k in range(KD)]
                        for k in range(KD):
                            nc.sync.dma_start(e2t[k][:], e2raw_d[b, k])
                        # per-block rows at partition base 0 (SBUF->SBUF DMA)
                        rstd_row = enc3.tile([1, EPB], bf16, tag="rstd_row")
                        nc.sync.dma_start(rstd_row[:], rstd16[b:b + 1, :])
                        nmu_row = enc3.tile([1, EPB], bf16, tag="nmu_row")
                        nc.sync.dma_start(nmu_row[:], nmu16[b:b + 1, :])
                        for s0, w in slabs:
                            # rstd/nmu broadcast across partitions, shared by
                            # all 3 feature chunks; g/beta applied per chunk
                            # as per-partition scalars
                            pA = pc3a.tile([P, SLAB], f32, tag="pA")
                            nc.tensor.matmul(pA[:, :w], ones_row_b[:, :],
                                             rstd_row[:, s0:s0 + w],
                                             start=True, stop=True)
                            pB = pc3b.tile([P, SLAB], f32, tag="pB")
                            nc.tensor.matmul(pB[:, :w], ones_row_b[:, :],
                                             nmu_row[:, s0:s0 + w],
                                             start=True, stop=True)
                            for m in range(KD):
                                t1 = enc3.tile([P, SLAB], f32, tag="t1")
                                nc.vector.tensor_tensor(out=t1[:, :w],
                                                        in0=e2t[m][:, s0:s0 + w],
                                                        in1=pA[:, :w], op=OP.mult)
                                nc.vector.tensor_tensor(out=t1[:, :w],
                                                        in0=t1[:, :w],
                                                        in1=pB[:, :w], op=OP.add)
                                nc.vector.tensor_scalar(
                                    out=wout[m][:, s0:s0 + w], in0=t1[:, :w],
                                    scalar1=eg_sb[:, m:m + 1],
                                    scalar2=ebeta_sb[:, m:m + 1],
                                    op0=OP.mult, op1=OP.add)
                        for k in range(KD):
                            nc.sync.dma_start(encT_d[b, k], wout[k][:])

            # =========================================================
            # Main layers
            # =========================================================
            with (
                tc.tile_pool(name="xrpool", bufs=1) as xrpool,
                tc.tile_pool(name="htpool", bufs=1) as htpool,
                tc.tile_pool(name="lw", bufs=2) as lw,
                tc.tile_pool(name="lep", bufs=1) as lep,
                tc.tile_pool(name="lsb", bufs=2) as lsb,
                tc.tile_pool(name="gat", bufs=2) as gat,
                tc.tile_pool(name="eetp", bufs=2) as eetp,
                tc.tile_pool(name="lps", bufs=3, space="PSUM") as lps,
                tc.tile_pool(name="lpt", bufs=2, space="PSUM") as lpt,
                tc.tile_pool(name="lpo", bufs=2, space="PSUM") as lpo,
            ):
                xr_sb = [xrpool.tile([P, D], bf16, tag=f"xr{b}", name=f"xr{b}")
                         for b in range(NBLK)]
                hT_sb = [[htpool.tile([P, P], bf16, tag=f"hT{b}_{k}",
                                      name=f"hT{b}_{k}") for k in range(KD)]
                         for b in range(NBLK)]
                for layer in range(L):
                    # ---- layer weights ----
                    Wl_sb = [lw.tile([P, D], bf16, tag=f"Wl{k}", name=f"Wl{k}")
                             for k in range(KD)]
                    Wr_sb = [lw.tile([P, D], bf16, tag=f"Wr{k}", name=f"Wr{k}")
                             for k in range(KD)]
                    We_sb = [lw.tile([P, D], bf16, tag=f"We{k}", name=f"We{k}")
                             for k in range(KD)]
                    for k in range(KD):
                        nc.sync.dma_start(Wl_sb[k][:], Wl_d[layer, k * P:(k + 1) * P, :])
                        nc.sync.dma_start(Wr_sb[k][:], Wr_d[layer, k * P:(k + 1) * P, :])
                        nc.sync.dma_start(We_sb[k][:], We_d[layer, k * P:(k + 1) * P, :])
                    bl_sb = lw.tile([1, D], bf16, tag="bl")
                    nc.sync.dma_start(bl_sb[:], bl_d[layer])
                    br_sb = lw.tile([1, D], bf16, tag="br")
                    nc.sync.dma_start(br_sb[:], br_d[layer])
                    attb_sb = lw.tile([P, D], bf16, tag="attb")
                    nc.sync.dma_start(attb_sb[:], att_b_d[layer])
                    bnsc_sb = lw.tile([P, D], bf16, tag="bnsc")
                    nc.sync.dma_start(bnsc_sb[:], bnsc_b_d[layer])
                    bnsh_sb = lw.tile([P, D], bf16, tag="bnsh")
                    nc.sync.dma_start(bnsh_sb[:], bnsh_b_d[layer])

                    # ---- stage D part 1: hT + xl for every block ----
                    for b in range(NBLK):
                        for k in range(KD):
                            pt = lpt.tile([P, P], f32, tag="pt")
                            nc.tensor.transpose(pt[:], h_sb[b][:, k * P:(k + 1) * P], ident[:])
                            nc.vector.tensor_copy(hT_sb[b][k][:], pt[:])
                        pxl = lps.tile([P, D], f32, tag="ps")
                        for k in range(KD):
                            nc.tensor.matmul(pxl[:], hT_sb[b][k][:], Wl_sb[k][:],
                                             start=(k == 0), stop=False)
                        nc.tensor.matmul(pxl[:], ones_row_b[:, :P], bl_sb[:],
                                         start=False, stop=True)
                        xl_t = lsb.tile([P, D], bf16, tag="xl_t")
                        nc.vector.tensor_copy(xl_t[:], pxl[:])
                        nc.sync.dma_start(xl_shard_d[b * P:(b + 1) * P, :], xl_t[:])

                    # ---- AllGather xl (xr compute overlaps with it) ----
                    nc.gpsimd.collective_compute(
                        "AllGather", OP.bypass, replica_groups=rg,
                        ins=[xl_shard_d[:, :]], outs=[xl_full_d[:, :]],
                    )

                    # ---- stage D part 2: xr for every block ----
                    for b in range(NBLK):
                        pxr = lps.tile([P, D], f32, tag="ps")
                        for k in range(KD):
                            nc.tensor.matmul(pxr[:], hT_sb[b][k][:], Wr_sb[k][:],
                                             start=(k == 0), stop=False)
                        nc.tensor.matmul(pxr[:], ones_row_b[:, :P], br_sb[:],
                                         start=False, stop=True)
                        nc.vector.tensor_copy(xr_sb[b][:], pxr[:])

                    # ---- stage E: edge message passing ----
                    eo_all = lep.tile([P, NBLK, D + H], bf16, tag="eo_all")
                    for b in range(NBLK):
                        gix = gat.tile([P, EPB // 16], i16, tag="gix")
                        nc.sync.dma_start(gix[:], gidx_d[b])
                        ohs_t = gat.tile([P, CPB, P], bf16, tag="ohs_t")
                        nc.sync.dma_start(ohs_t[:], ohs_d[b])
                        ohg_t = gat.tile([P, CPB, P], bf16, tag="ohg_t")
                        nc.sync.dma_start(ohg_t[:], ohg_d[b])
                        eet = [eetp.tile([P, EPB], bf16, tag=f"eet{k}", name=f"eet{k}")
                               for k in range(KD)]
                        for k in range(KD):
                            nc.sync.dma_start(eet[k][:], encT_d[b, k])
                        xsg = eetp.tile([P, CPB, D], bf16, tag="xsg")
                        nc.gpsimd.dma_gather(xsg[:], xl_full_d[:, :], gix[:], EPB, EPB, D,
                                             single_packet=False)
                        psum_o = lpo.tile([P, D + H], f32, tag="po")
                        for c in range(CPB):
                            xsrc = xsg[:, c]
                            psum_s = lps.tile([P, D], f32, tag="ps")
                            for k in range(KD):
                                nc.tensor.matmul(psum_s[:], eet[k][:, c * P:(c + 1) * P],
                                                 We_sb[k][:], start=(k == 0), stop=False)
                            nc.tensor.matmul(psum_s[:], ohg_t[:, c, :], xr_sb[b][:],
                                             start=False, stop=True)
                            s_sb = lsb.tile([P, D], bf16, tag="s_sb")
                            nc.vector.tensor_tensor(out=s_sb[:], in0=psum_s[:],
                                                    in1=xsrc, op=OP.add)
                            m_sb = lsb.tile([P, D], bf16, tag="m_sb")
                            if HW_ACTS:
                                nc.scalar.activation(m_sb[:], s_sb[:], AF.Prelu, alpha=0.2)
                            else:
                                nc.scalar.activation(m_sb[:], s_sb[:], AF.Relu)
                            t_sb = lsb.tile([P, D], bf16, tag="t_sb")
                            nc.vector.tensor_tensor(out=t_sb[:], in0=m_sb[:], in1=attb_sb[:],
                                                    op=OP.mult)
                            lg = lsb.tile([P, H], f32, tag="lg")
                            nc.vector.tensor_reduce(
                                out=lg[:], in_=t_sb[:].rearrange("p (h c) -> p h c", h=H),
                                axis=mybir.AxisListType.X, op=OP.add)
                            z_sb = lsb.tile([P, D + H], bf16, tag="z_sb")
                            nc.scalar.activation(z_sb[:, D:], lg[:], AF.Exp)
                            el_b = z_sb[:, D:].rearrange("p (h o) -> p h o", o=1).to_broadcast([P, H, C])
                            nc.vector.tensor_tensor(
                                out=z_sb[:, :D].rearrange("p (h c) -> p h c", h=H),
                                in0=xsrc.rearrange("p (h c) -> p h c", h=H),
                                in1=el_b, op=OP.mult)
                            nc.tensor.matmul(psum_o[:], ohs_t[:, c, :], z_sb[:],
                                             start=(c == 0), stop=(c == CPB - 1))
                        # stash numerators+denominators; epilogue is batched
                        nc.scalar.copy(eo_all[:, b, :], psum_o[:])

                    # ---- batched epilogue for all blocks ----
                    den_f = lep.tile([P, NBLK, H], f32, tag="den_f")
                    nc.vector.tensor_scalar(out=den_f[:], in0=eo_all[:, :, D:],
                                            scalar1=1e-16, scalar2=None, op0=OP.add)
                    rec_f = lep.tile([P, NBLK, H], f32, tag="rec_f")
                    nc.vector.reciprocal_approx_fast(rec_f[:], den_f[:])
                    rec_b16 = lep.tile([P, NBLK, H], bf16, tag="rec_b16")
                    nc.vector.tensor_copy(rec_b16[:], rec_f[:])
                    num_v = eo_all[:, :, :D].rearrange("p n (h c) -> p n h c", h=H)
                    rec_v = rec_b16[:].rearrange("p n (h o) -> p n h o", o=1) \
                        .to_broadcast([P, NBLK, H, C])
                    nc.vector.tensor_tensor(out=num_v, in0=num_v, in1=rec_v,
                                            op=OP.mult)
                    o1_v = eo_all[:, :, :D]
                    bnsc_v = bnsc_sb[:].rearrange("p (o d) -> p o d", o=1) \
                        .to_broadcast([P, NBLK, D])
                    bnsh_v = bnsh_sb[:].rearrange("p (o d) -> p o d", o=1) \
                        .to_broadcast([P, NBLK, D])
                    nc.vector.tensor_tensor(out=o1_v, in0=o1_v, in1=bnsc_v, op=OP.mult)
                    nc.vector.tensor_tensor(out=o1_v, in0=o1_v, in1=bnsh_v, op=OP.add)
                    ex_f = lep.tile([P, NBLK, D], f32, tag="ex_f")
                    nc.scalar.activation(ex_f[:], o1_v, AF.Exp, scale=-1.0)
                    nc.vector.tensor_scalar(out=ex_f[:], in0=ex_f[:], scalar1=1.0,
                                            scalar2=None, op0=OP.add)
                    nc.vector.reciprocal_approx_fast(ex_f[:], ex_f[:])
                    nc.vector.tensor_tensor(out=o1_v, in0=o1_v, in1=ex_f[:],
                                            op=OP.mult)
                    for b in range(NBLK):
                        nc.vector.tensor_tensor(out=h_sb[b][:], in0=h_sb[b][:],
                                                in1=eo_all[:, b, :D], op=OP.add)

            # =========================================================
            # Stage F: pooling + head
            # =========================================================
            with (
                tc.tile_pool(name="fsb", bufs=3) as fsb,
                tc.tile_pool(name="fkeep", bufs=1) as fkeep,
                tc.tile_pool(name="fps", bufs=2, space="PSUM") as fps,
                tc.tile_pool(name="fsum", bufs=1, space="PSUM") as fsum,
            ):
                psum_sum = fsum.tile([G, D], f32, tag="psum_sum")
                psum_cnt = fsum.tile([1, G], f32, tag="psum_cnt")
                bm = [fkeep.tile([P, MAXG * NBLK], f32, tag=f"bm{k}", name=f"bm{k}")
                      for k in range(KD)]
                for b in range(NBLK):
                    ohb = fsb.tile([P, G], f32, tag="ohb")
                    nc.sync.dma_start(ohb[:], oh_d[b])
                    mab = fsb.tile([P, MAXG], f32, tag="mab")
                    nc.sync.dma_start(mab[:], maskAB_d[b])
                    nc.tensor.matmul(psum_sum[:], ohb[:], h_sb[b][:],
                                     start=(b == 0), stop=(b == NBLK - 1))
                    nc.tensor.matmul(psum_cnt[:], ones_col[:, :1], ohb[:],
                                     start=(b == 0), stop=(b == NBLK - 1))
                    for half in range(MAXG):
                        mh = fsb.tile([P, D], f32, tag="mh")
                        nc.vector.tensor_scalar(out=mh[:], in0=h_sb[b][:],
                                                scalar1=mab[:, half:half + 1], scalar2=None,
                                                op0=OP.add)
                        for k in range(KD):
                            pt = fps.tile([P, P], f32, tag="pt")
                            nc.tensor.transpose(pt[:], mh[:, k * P:(k + 1) * P], ident[:])
                            mt = fsb.tile([P, P], f32, tag="mt")
                            nc.vector.tensor_copy(mt[:], pt[:])
                            nc.vector.tensor_reduce(
                                out=bm[k][:, MAXG * b + half:MAXG * b + half + 1],
                                in_=mt[:], axis=mybir.AxisListType.X, op=OP.max)
                # combine per-graph maxes
                gmaxT = [fkeep.tile([P, G], f32, tag=f"gmaxT{k}", name=f"gmaxT{k}")
                         for k in range(KD)]
                for g in range(G):
                    cr = fsb.tile([1, MAXG * NBLK], f32, tag="cr")
                    nc.sync.dma_start(cr[:], cmb_d[g])
                    pc = fps.tile([P, MAXG * NBLK], f32, tag="pt")
                    nc.tensor.matmul(pc[:], ones_row[:, :P], cr[:], start=True, stop=True)
                    for k in range(KD):
                        mm = fsb.tile([P, MAXG * NBLK], f32, tag="mm")
                        nc.vector.tensor_tensor(out=mm[:], in0=bm[k][:], in1=pc[:], op=OP.add)
                        nc.vector.tensor_reduce(out=gmaxT[k][:, g:g + 1], in_=mm[:],
                                                axis=mybir.AxisListType.X, op=OP.max)
                # partial sums to DRAM
                sum_sb = fsb.tile([G, D], f32, tag="sum_sb")
                nc.vector.tensor_copy(sum_sb[:], psum_sum[:])
                for k in range(KD):
                    pt = fps.tile([P, G], f32, tag="pt")
                    nc.tensor.transpose(pt[:, :G], sum_sb[:, k * P:(k + 1) * P], ident[:G, :G])
                    st = fsb.tile([P, G], f32, tag="st")
                    nc.vector.tensor_copy(st[:], pt[:, :G])
                    nc.sync.dma_start(pool_part_d[k * P:(k + 1) * P, :], st[:])
                    nc.sync.dma_start(pool_part_d[D + k * P:D + (k + 1) * P, :], gmaxT[k][:])
                cntT = fsb.tile([1, G], f32, tag="cntT")
                nc.vector.tensor_copy(cntT[:], psum_cnt[:])
                nc.sync.dma_start(pool_part_d[2 * D:2 * D + 1, :], cntT[:])

                # ---- tiny AllGather of partials ----
                nc.gpsimd.collective_compute(
                    "AllGather", OP.bypass, replica_groups=rg,
                    ins=[pool_part_d[:, :]], outs=[pool_all_d[:, :]],
                )

                # ---- combine + head (replicated on all devices) ----
                n_dev_ = n_dev
                STRIDE = 2 * D + 1
                meanT = [fkeep.tile([P, G], f32, tag=f"meanT{k}", name=f"meanT{k}")
                         for k in range(KD)]
                maxT = [fkeep.tile([P, G], f32, tag=f"maxT{k}", name=f"maxT{k}")
                        for k in range(KD)]
                cnt_tot = fkeep.tile([1, G], f32, tag="cnt_tot")
                for dv in range(n_dev_):
                    base = dv * STRIDE
                    for k in range(KD):
                        ts = fsb.tile([P, G], f32, tag="ts")
                        nc.sync.dma_start(ts[:], pool_all_d[base + k * P:base + (k + 1) * P, :])
                        tm = fsb.tile([P, G], f32, tag="tm")
                        nc.sync.dma_start(tm[:], pool_all_d[base + D + k * P:base + D + (k + 1) * P, :])
                        if dv == 0:
                            nc.vector.tensor_copy(meanT[k][:], ts[:])
                            nc.vector.tensor_copy(maxT[k][:], tm[:])
                        else:
                            nc.vector.tensor_tensor(out=meanT[k][:], in0=meanT[k][:],
                                                    in1=ts[:], op=OP.add)
                            nc.vector.tensor_tensor(out=maxT[k][:], in0=maxT[k][:],
                                                    in1=tm[:], op=OP.max)
                    tc_ = fsb.tile([1, G], f32, tag="tc_")
                    nc.sync.dma_start(tc_[:], pool_all_d[base + 2 * D:base + 2 * D + 1, :])
                    if dv == 0:
                        nc.vector.tensor_copy(cnt_tot[:], tc_[:])
                    else:
                        nc.vector.tensor_tensor(out=cnt_tot[:], in0=cnt_tot[:], in1=tc_[:],
                                                op=OP.add)
                nc.vector.tensor_scalar(out=cnt_tot[:], in0=cnt_tot[:], scalar1=1.0,
                                        scalar2=None, op0=OP.max)
                inv_cnt = fkeep.tile([1, G], f32, tag="inv_cnt")
                nc.vector.reciprocal(inv_cnt[:], cnt_tot[:])
                pic = fps.tile([P, G], f32, tag="pt")
                nc.tensor.matmul(pic[:], ones_row[:, :P], inv_cnt[:], start=True, stop=True)
                for k in range(KD):
                    nc.vector.tensor_tensor(out=meanT[k][:], in0=meanT[k][:], in1=pic[:],
                                            op=OP.mult)
                hgT = meanT + maxT          # 6 k-tiles of [128, G] = hg transposed

                # head weights
                pW_sb = [fkeep.tile([P, D], f32, tag=f"pW{k}", name=f"pW{k}")
                         for k in range(2 * KD)]
                for k in range(2 * KD):
                    nc.sync.dma_start(pW_sb[k][:], pW_d[k * P:(k + 1) * P, :])
                pb_sb = fkeep.tile([1, D], f32, tag="pb")
                nc.sync.dma_start(pb_sb[:], pb_d[:, :])
                hW1_sb = [fkeep.tile([P, D], f32, tag=f"hW1_{k}", name=f"hW1_{k}")
                          for k in range(KD)]
                for k in range(KD):
                    nc.sync.dma_start(hW1_sb[k][:], hW1_d[k * P:(k + 1) * P, :])
                hb1_sb = fkeep.tile([1, D], f32, tag="hb1")
                nc.sync.dma_start(hb1_sb[:], hb1_d[:, :])
                hW2_sb = [fkeep.tile([P, D // 2], f32, tag=f"hW2_{k}", name=f"hW2_{k}")
                          for k in range(KD)]
                for k in range(KD):
                    nc.sync.dma_start(hW2_sb[k][:], hW2_d[k * P:(k + 1) * P, :])
                hb2_sb = fkeep.tile([1, D // 2], f32, tag="hb2")
                nc.sync.dma_start(hb2_sb[:], hb2_d[:, :])
                hW3_sb = fkeep.tile([P, 2], f32, tag="hW3")
                nc.sync.dma_start(hW3_sb[:], hW3_d[:, :].rearrange("(k p) o -> p (k o)", p=P))
                hb3_sb = fkeep.tile([1, 1], f32, tag="hb3")
                nc.sync.dma_start(hb3_sb[:], hb3_d[:, :])

                def mlp_layer(in_tiles, W_tiles, b_row, out_feats, lid, act=True):
                    outs = []
                    n_out_tiles = (out_feats + P - 1) // P
                    for m in range(n_out_tiles):
                        mw = min(P, out_feats - m * P)
                        pm = fps.tile([P, G], f32, tag="ph", name=f"ph{lid}_{m}")
                        for k, (it, wt) in enumerate(zip(in_tiles, W_tiles)):
                            nc.tensor.matmul(pm[:mw, :], wt[:, m * P:m * P + mw], it[:],
                                             start=(k == 0), stop=False)
                        nc.tensor.matmul(pm[:mw, :], b_row[:, m * P:m * P + mw],
                                         ones_row[:, :G], start=False, stop=True)
                        ot = fkeep.tile([P, G], f32, tag=f"ot{lid}_{m}", name=f"ot{lid}_{m}")
                        if mw < P:
                            nc.vector.memset(ot[mw:, :], 0.0)
                        if act:
                            emit_silu(fsb, ot[:mw, :], pm[:mw, :], [mw, G])
                        else:
                            nc.vector.tensor_copy(ot[:mw, :], pm[:mw, :])
                        outs.append(ot)
                    return outs

                h1 = mlp_layer(hgT, pW_sb, pb_sb, D, 1)
                h2 = mlp_layer(h1, hW1_sb, hb1_sb, D, 2)
                h3 = mlp_layer(h2, hW2_sb, hb2_sb, D // 2, 3)
                pf = fps.tile([1, G], f32, tag="pf")
                nc.tensor.matmul(pf[:], hW3_sb[:, 0:1], h3[0][:], start=True, stop=False)
                nc.tensor.matmul(pf[:], hW3_sb[:, 1:2], h3[1][:], start=False, stop=False)
                nc.tensor.matmul(pf[:], hb3_sb[:, :1], ones_row[:, :G], start=False, stop=True)
                fo = fsb.tile([1, G], f32, tag="fo")
                nc.vector.tensor_copy(fo[:], pf[:])
                nc.sync.dma_start(out_d[:].rearrange("(o g) -> o g", o=1), fo[:])

    nc.compile()
    return nc


# --------------------------------------------------------------------------
# entry point
# --------------------------------------------------------------------------

def kernel(**inputs):
    n_dev = 8
    meta, rep, devs = prep_host(inputs, n_dev)
    nc = build_program(meta)

    in_maps = []
    for d in range(n_dev):
        m = dict(rep)
        m.update(devs[d])
        in_maps.append(m)

    global LAST_RESULTS
    res = run_bass_kernel_spmd(nc, in_maps, core_ids=list(range(n_dev)),
                               trace=TRACE)
    LAST_RESULTS = res
    out = np.asarray(res.results[0]["out"], np.float32)
    return out


# revision 23
# speedup vs baseline: 1.6534x; 1.0390x over previous
"""Trainium2 Bass kernel for nn_EquivariantProteinGNN (GATv2-style message passing).

Strategy (8 NeuronCores, SPMD):
  - Nodes padded to 20480 and split into 8 contiguous shards of 2560 (20 blocks
    of 128). Edges assigned to the device owning their dst node, sorted by dst,
    and packed into fixed-size per-block runs (CPB chunks of 128 edge slots,
    dummy slots excluded via host-precomputed one-hot matrices).
  - Edge encoder (stage C) runs in transposed space with 512-edge slabs:
    RBF built by a broadcast matmul + Square/Exp activations, eb1 folded via a
    ones-row in rbfT, LayerNorm stats accumulated for ALL blocks into one PSUM
    tile via selector matmuls, then a tiny batched stats pass and a per-block
    normalize pass (raw e2 round-trips DRAM in bf16).
  - Per layer: xl for all blocks -> AllGather (overlapped with xr compute),
    then per-block edge processing: segment softmax and message scatter via
    one-hot matmuls accumulating in PSUM; one-hots come from the host.
  - Pooling: per-graph sums via one-hot matmul, maxes via masked transposed
    reduces; tiny AllGather of partials; head MLP replicated.
"""

import math
import ml_dtypes
import numpy as np

import concourse.bass as bass
import concourse.bacc as bacc
import concourse.mybir as mybir
import concourse.tile as tile
from concourse.bass_utils import run_bass_kernel_spmd
from concourse.masks import make_identity
from concourse.library_config import mlp as mlp_lib

P = 128
D = 384
H, C = 12, 32
NUM_RBF = 100
RBF_MIN, RBF_MAX = 0.0, 30.0
NEG_BIG = -1.0e30
SLAB = 512

f32 = mybir.dt.float32
bf16 = mybir.dt.bfloat16
i32 = mybir.dt.int32
i16 = mybir.dt.int16
AF = mybir.ActivationFunctionType
OP = mybir.AluOpType

HW_ACTS = True

TRACE = False
LAST_RESULTS = None


# --------------------------------------------------------------------------
# host-side preprocessing
# --------------------------------------------------------------------------

def prep_host(inputs, n_dev=8, G=32):
    x = np.asarray(inputs["x"], np.float32)
    pos = np.asarray(inputs["pos"], np.float32)
    edge_index = np.asarray(inputs["edge_index"], np.int64)
    batch = np.asarray(inputs["batch"], np.int64)

    N = x.shape[0]
    E = edge_index.shape[1]
    L = np.asarray(inputs["Wl"]).shape[0]

    PD = int(math.ceil(N / (n_dev * P))) * P          # nodes per device (padded)
    N_pad = PD * n_dev
    NBLK = PD // P

    src = edge_index[0].astype(np.int64)
    dst = edge_index[1].astype(np.int64)

    # edges per 128-node block
    blk = dst // P
    cnt = np.bincount(blk, minlength=N_pad // P)
    CPB = int(math.ceil(cnt.max() / P))
    EPB = CPB * P

    # slot edges: per global block, a run of EPB slots
    order = np.argsort(dst, kind="stable")
    src_s, dst_s = src[order], dst[order]
    blk_s = dst_s // P
    start = np.zeros(len(cnt), np.int64)
    start[1:] = np.cumsum(cnt)[:-1]
    within = np.arange(E) - start[blk_s]
    slot = blk_s * EPB + within                       # global slot id

    n_slots = (N_pad // P) * EPB
    g_src = np.zeros(n_slots, np.int64)
    g_dstrel = np.full(n_slots, -1, np.int64)
    g_dist = np.zeros(n_slots, np.float32)
    g_src[slot] = src_s
    g_dstrel[slot] = dst_s - blk_s * P
    g_dist[slot] = np.linalg.norm(pos[src_s] - pos[dst_s], axis=-1)

    qq = np.arange(P, dtype=np.int64)

    devs = []
    SPD = NBLK * EPB                                  # slots per device
    for d in range(n_dev):
        sl = slice(d * SPD, (d + 1) * SPD)
        gsr = g_src[sl].astype(np.int16).reshape(NBLK, EPB)
        gidx = np.tile(gsr.reshape(NBLK, EPB // 16, 16).transpose(0, 2, 1), (1, 8, 1)).copy()
        dr = g_dstrel[sl].reshape(NBLK, CPB, P)       # [b, c, p]
        # scatter one-hot: ohs[b, p, c, q] = (dr[b, c, p] == q)
        ohs = (dr[..., None] == qq).astype(ml_dtypes.bfloat16).transpose(0, 2, 1, 3).copy()
        # gather one-hot: ohg[b, p, c, q] = (dr[b, c, q] == p)
        ohg = (dr[..., None] == qq).astype(ml_dtypes.bfloat16).transpose(0, 3, 1, 2).copy()
        distT = g_dist[sl].reshape(NBLK, 1, EPB).astype(ml_dtypes.bfloat16)

        # node features, transposed for the embedding matmul
        xdev = np.zeros((PD, x.shape[1]), np.float32)
        lo, hi = d * PD, min((d + 1) * PD, N)
        if hi > lo:
            xdev[: hi - lo] = x[lo:hi]
        xT = np.ascontiguousarray(xdev.T).astype(ml_dtypes.bfloat16)  # (20, PD)

        # pooling helpers
        bdev = np.full(PD, -1, np.int64)
        if hi > lo:
            bdev[: hi - lo] = batch[lo:hi]
        oh = np.zeros((PD, G), np.float32)
        real = bdev >= 0
        oh[np.arange(PD)[real], bdev[real]] = 1.0
        oh = oh.reshape(NBLK, P, G)

        devs.append(dict(gidx=gidx, ohs=ohs, ohg=ohg, distT=distT,
                         xT=xT, oh=oh, bdev=bdev))

    # pooling masks: per block, up to MAXG distinct graphs
    MAXG = 1
    for dv in devs:
        bdev = dv["bdev"]
        for b in range(NBLK):
            u = np.unique(bdev[b * P:(b + 1) * P])
            MAXG = max(MAXG, len(u[u >= 0]))
    for dv in devs:
        bdev = dv.pop("bdev")
        maskG = np.full((NBLK, P, MAXG), NEG_BIG, np.float32)
        cmb = np.full((G, MAXG * NBLK), NEG_BIG, np.float32)
        for b in range(NBLK):
            bb = bdev[b * P:(b + 1) * P]
            u = np.unique(bb)
            u = u[u >= 0]
            for mi, g in enumerate(u):
                maskG[b, :, mi] = np.where(bb == g, 0.0, NEG_BIG)
                cmb[g, MAXG * b + mi] = 0.0
        dv["maskAB"] = maskG
        dv["cmb"] = cmb.reshape(G, 1, MAXG * NBLK)

    # replicated parameter pack
    def bc(v):                                        # [128, n] broadcast
        v = np.asarray(v, np.float32).reshape(1, -1)
        return np.ascontiguousarray(np.broadcast_to(v, (P, v.shape[1])))

    def row(v):
        return np.asarray(v, np.float32).reshape(1, -1)

    def b16(v):
        return np.asarray(v, np.float32).astype(ml_dtypes.bfloat16)

    bn_scale = (np.asarray(inputs["bn_g"], np.float32)
                / np.sqrt(np.asarray(inputs["bn_v"], np.float32) + 1e-5))
    bn_shift = (np.asarray(inputs["bn_b"], np.float32)
                + (np.asarray(inputs["cb"], np.float32)
                   - np.asarray(inputs["bn_m"], np.float32)) * bn_scale)

    centers = np.linspace(RBF_MIN, RBF_MAX, NUM_RBF).astype(np.float32)
    spacing = (RBF_MAX - RBF_MIN) / (NUM_RBF - 1)
    gamma = 1.0 / (spacing ** 2 + 1e-8)

    att = np.asarray(inputs["att"], np.float32).reshape(L, 1, D)
    att_b = np.ascontiguousarray(np.broadcast_to(att, (L, P, D)))
    bnsc_b = np.ascontiguousarray(np.broadcast_to(bn_scale.reshape(L, 1, D), (L, P, D)))
    bnsh_b = np.ascontiguousarray(np.broadcast_to(bn_shift.reshape(L, 1, D), (L, P, D)))

    # eW1 with eb1 folded as an extra contraction row
    eW1s = np.vstack([np.asarray(inputs["eW1"], np.float32),
                      np.asarray(inputs["eb1"], np.float32).reshape(1, D)])

    rep = dict(
        emb_W=b16(inputs["emb_W"]),
        emb_b=b16(row(inputs["emb_b"])),
        emb_g_b=bc(np.asarray(inputs["emb_g"]) * 0.5),
        emb_beta_b=bc(np.asarray(inputs["emb_beta"]) * 0.5),
        eW1s=b16(eW1s),
        eW2=b16(np.asarray(inputs["eW2"]) * 0.5),
        eb2=b16(row(inputs["eb2"])),
        e_g_col=np.asarray(inputs["e_g"], np.float32).reshape(-1, P).T.copy(),
        e_beta_col=np.asarray(inputs["e_beta"], np.float32).reshape(-1, P).T.copy(),
        neg_centers=-centers.reshape(NUM_RBF, 1),
        iota2n=np.ascontiguousarray(np.broadcast_to(
            np.arange(2 * NBLK, dtype=np.float32), (P, 2 * NBLK))),
        Wl=b16(inputs["Wl"]), bl=b16(np.asarray(inputs["bl"]).reshape(L, 1, D)),
        Wr=b16(inputs["Wr"]), br=b16(np.asarray(inputs["br"]).reshape(L, 1, D)),
        We=b16(inputs["We"]),
        att_b=b16(att_b), bnsc_b=b16(bnsc_b * 0.5), bnsh_b=b16(bnsh_b * 0.5),
        pW=np.asarray(inputs["pW"], np.float32) * 0.5,
        pb=row(inputs["pb"]) * 0.5,
        hW1=np.asarray(inputs["hW1"], np.float32) * 0.5,
        hb1=row(inputs["hb1"]) * 0.5,
        hW2=np.asarray(inputs["hW2"], np.float32) * 0.5,
        hb2=row(inputs["hb2"]) * 0.5,
        hW3=np.pad(np.asarray(inputs["hW3"], np.float32), ((0, 64), (0, 0))).reshape(2, P).T.copy(),
        hb3=row(inputs["hb3"]),
    )

    meta = dict(n_dev=n_dev, N=N, E=E, G=G, L=L, PD=PD, N_pad=N_pad,
                NBLK=NBLK, CPB=CPB, EPB=EPB, gamma=gamma,
                x_in=x.shape[1], MAXG=MAXG)
    return meta, rep, devs


# --------------------------------------------------------------------------
# device program
# --------------------------------------------------------------------------

def build_program(meta):
    n_dev = meta["n_dev"]
    L, G = meta["L"], meta["G"]
    PD, N_pad = meta["PD"], meta["N_pad"]
    NBLK, CPB, EPB = meta["NBLK"], meta["CPB"], meta["EPB"]
    MAXG = meta["MAXG"]
    gamma = meta["gamma"]
    XIN = meta["x_in"]
    KD = D // P                                        # 3 feature k-chunks
    slabs = [(s, min(SLAB, EPB - s)) for s in range(0, EPB, SLAB)]

    nc = bacc.Bacc(None, target_bir_lowering=False, debug=False)

    # ---- I/O ----
    def inp(name, shape, dtype=f32):
        return nc.dram_tensor(name, list(shape), dtype, kind="ExternalInput")

    gidx_d = inp("gidx", (NBLK, P, EPB // 16), i16)
    ohs_d = inp("ohs", (NBLK, P, CPB, P), bf16)
    ohg_d = inp("ohg", (NBLK, P, CPB, P), bf16)
    distT_d = inp("distT", (NBLK, 1, EPB), bf16)
    xT_d = inp("xT", (XIN, PD), bf16)
    oh_d = inp("oh", (NBLK, P, G))
    maskAB_d = inp("maskAB", (NBLK, P, MAXG))
    cmb_d = inp("cmb", (G, 1, MAXG * NBLK))

    emb_W_d = inp("emb_W", (XIN, D), bf16)
    emb_b_d = inp("emb_b", (1, D), bf16)
    emb_g_b_d = inp("emb_g_b", (P, D))
    emb_beta_b_d = inp("emb_beta_b", (P, D))
    eW1s_d = inp("eW1s", (NUM_RBF + 1, D), bf16)
    eW2_d = inp("eW2", (D, D), bf16)
    eb2_d = inp("eb2", (1, D), bf16)
    e_g_col_d = inp("e_g_col", (P, KD))
    e_beta_col_d = inp("e_beta_col", (P, KD))
    neg_centers_d = inp("neg_centers", (NUM_RBF, 1))
    iota2n_d = inp("iota2n", (P, 2 * NBLK))
    Wl_d = inp("Wl", (L, D, D), bf16)
    bl_d = inp("bl", (L, 1, D), bf16)
    Wr_d = inp("Wr", (L, D, D), bf16)
    br_d = inp("br", (L, 1, D), bf16)
    We_d = inp("We", (L, D, D), bf16)
    att_b_d = inp("att_b", (L, P, D), bf16)
    bnsc_b_d = inp("bnsc_b", (L, P, D), bf16)
    bnsh_b_d = inp("bnsh_b", (L, P, D), bf16)
    pW_d = inp("pW", (2 * D, D))
    pb_d = inp("pb", (1, D))
    hW1_d = inp("hW1", (D, D))
    hb1_d = inp("hb1", (1, D))
    hW2_d = inp("hW2", (D, D // 2))
    hb2_d = inp("hb2", (1, D // 2))
    hW3_d = inp("hW3", (P, 2))
    hb3_d = inp("hb3", (1, 1))

    out_d = nc.dram_tensor("out", [G], f32, kind="ExternalOutput")

    # internal DRAM
    e2raw_d = nc.dram_tensor("e2raw", [NBLK, KD, P, EPB], bf16)
    encT_d = nc.dram_tensor("encT", [NBLK, KD, P, EPB], bf16)
    stats_d = nc.dram_tensor("stats", [2 * NBLK, EPB], f32)
    xl_shard_d = nc.dram_tensor("xl_shard", [PD, D], bf16)
    shared_as = "Shared" if n_dev > 4 else "Local"
    xl_full_d = nc.dram_tensor("xl_full", [N_pad, D], bf16, addr_space=shared_as)
    pool_part_d = nc.dram_tensor("pool_part", [2 * D + 1, G], f32)
    pool_all_d = nc.dram_tensor("pool_all", [n_dev * (2 * D + 1), G], f32, addr_space=shared_as)

    rg = [list(range(n_dev))]

    with tile.TileContext(nc) as tc:
        with (
            tc.tile_pool(name="consts", bufs=1) as consts,
            tc.tile_pool(name="hpool", bufs=1) as hpool,
        ):
            nc.gpsimd.load_library(mlp_lib)
            ident = consts.tile([P, P], f32, tag="ident")
            make_identity(nc, ident)
            ones_row = consts.tile([1, P], f32, tag="ones_row")
            nc.vector.memset(ones_row[:], 1.0)
            ones_col = consts.tile([P, 1], f32, tag="ones_col")
            nc.vector.memset(ones_col[:], 1.0)
            ones_row_b = consts.tile([1, P], bf16, tag="ones_row_b")
            nc.vector.memset(ones_row_b[:], 1.0)
            ones_sq_b = consts.tile([P, SLAB], bf16, tag="ones_sq_b")
            nc.vector.memset(ones_sq_b[:], 1.0)
            ones_1r100 = consts.tile([1, NUM_RBF], bf16, tag="ones_1r100")
            nc.vector.memset(ones_1r100[:], 1.0)
            eps_col = consts.tile([P, 1], f32, tag="eps_col")
            nc.vector.memset(eps_col[:], 1e-5)

            silu_n = [0]

            def emit_silu(pool, out_ap, in_ap, shape):
                # in_ap must hold y/2 (0.5 folded into the preceding affine);
                # silu(y) = (y/2)*(1+tanh(y/2)) -> 1 ACT + 1 DVE op
                silu_n[0] += 1
                sn = silu_n[0]
                th = pool.tile(shape, bf16, tag="silu_th", name=f"silu_th{sn}")
                nc.scalar.activation(th[:], in_ap, AF.Tanh)
                nc.vector.scalar_tensor_tensor(out=out_ap, in0=th[:], scalar=1.0,
                                               in1=in_ap, op0=OP.add, op1=OP.mult)

            h_sb = [hpool.tile([P, D], f32, tag=f"h{b}", name=f"h{b}")
                    for b in range(NBLK)]

            # =========================================================
            # Stage B: node embedding  h0 = silu(LN(x @ emb_W + emb_b))
            # =========================================================
            with (
                tc.tile_pool(name="embsb", bufs=2) as embsb,
                tc.tile_pool(name="embc", bufs=1) as embc,
                tc.tile_pool(name="embxc", bufs=1) as embxc,
                tc.tile_pool(name="embps", bufs=2, space="PSUM") as embps,
            ):
                xT_sb = embc.tile([XIN, PD], bf16, tag="xT")
                nc.sync.dma_start(xT_sb[:], xT_d[:, :])
                embW_sb = embc.tile([XIN, D], bf16, tag="embW")
                nc.sync.dma_start(embW_sb[:], emb_W_d[:, :])
                embb_sb = embc.tile([1, D], bf16, tag="embb")
                nc.sync.dma_start(embb_sb[:], emb_b_d[:, :])
                emb_g_sb = embc.tile([P, D], f32, tag="embg")
                nc.sync.dma_start(emb_g_sb[:], emb_g_b_d[:, :])
                emb_beta_sb = embc.tile([P, D], f32, tag="embbeta")
                nc.sync.dma_start(emb_beta_sb[:], emb_beta_b_d[:, :])
                var_all = embc.tile([P, NBLK], f32, tag="var_all")
                rstd_all = embc.tile([P, NBLK], f32, tag="rstd_all")
                xc_all = [embxc.tile([P, D], f32, tag=f"xc{b}", name=f"xc{b}")
                          for b in range(NBLK)]

                # pass 1: matmul + center + accumulate var; Ln batched after
                for b in range(NBLK):
                    ps = embps.tile([P, D], f32, tag="ps")
                    nc.tensor.matmul(ps[:], xT_sb[:, b * P:(b + 1) * P], embW_sb[:],
                                     start=True, stop=False)
                    nc.tensor.matmul(ps[:], ones_row_b[:, :P], embb_sb[:],
                                     start=False, stop=True)
                    mu = embsb.tile([P, 1], f32, tag="mu")
                    nc.vector.tensor_reduce(out=mu[:], in_=ps[:],
                                            axis=mybir.AxisListType.X, op=OP.add)
                    nc.vector.tensor_scalar(out=mu[:], in0=mu[:], scalar1=1.0 / D,
                                            scalar2=None, op0=OP.mult)
                    xc = xc_all[b]
                    nc.vector.tensor_scalar(out=xc[:], in0=ps[:], scalar1=mu[:, :1],
                                            scalar2=None, op0=OP.subtract)
                    sq = embsb.tile([P, D], f32, tag="sq")
                    nc.scalar.activation(sq[:], xc[:], AF.Square,
                                         accum_out=var_all[:, b:b + 1])
                # one Ln/Exp pair for all blocks (avoids act-table thrash)
                nc.scalar.activation(rstd_all[:], var_all[:], AF.Ln, scale=1.0 / D,
                                     bias=eps_col[:, :1])
                nc.scalar.activation(rstd_all[:], rstd_all[:], AF.Exp, scale=-0.5)
                for b in range(NBLK):
                    xc = xc_all[b]
                    nc.vector.tensor_scalar(out=xc[:], in0=xc[:],
                                            scalar1=rstd_all[:, b:b + 1],
                                            scalar2=None, op0=OP.mult)
                    nc.vector.tensor_tensor(out=xc[:], in0=xc[:], in1=emb_g_sb[:], op=OP.mult)
                    nc.vector.tensor_tensor(out=xc[:], in0=xc[:], in1=emb_beta_sb[:], op=OP.add)
                    emit_silu(embsb, h_sb[b][:], xc[:], [P, D])

            # =========================================================
            # Stage C: edge encoder -> encT
            # =========================================================
            with tc.tile_pool(name="encstat", bufs=1) as encstat:
                eg_sb = encstat.tile([P, KD], f32, tag="eg")
                nc.sync.dma_start(eg_sb[:], e_g_col_d[:, :])
                ebeta_sb = encstat.tile([P, KD], f32, tag="ebeta")
                nc.sync.dma_start(ebeta_sb[:], e_beta_col_d[:, :])
                rstd16 = encstat.tile([NBLK, EPB], bf16, tag="rstd16")
                nmu16 = encstat.tile([NBLK, EPB], bf16, tag="nmu16")

                # ---- pass 1: raw e2 (pre-LN) in slabs + stats accumulation ----
                with (
                    tc.tile_pool(name="encw", bufs=1) as encw,
                    tc.tile_pool(name="encsb", bufs=3) as encsb,
                    tc.tile_pool(name="encrb", bufs=1) as encrb,
                    tc.tile_pool(name="pcrbf", bufs=1, space="PSUM") as pcrbf,
                    tc.tile_pool(name="pce1", bufs=2, space="PSUM") as pce1,
                    tc.tile_pool(name="pce2", bufs=2, space="PSUM") as pce2,
                    tc.tile_pool(name="pcstat", bufs=1, space="PSUM") as pcstat,
                ):
                    eW1s_sb = encw.tile([NUM_RBF + 1, D], bf16, tag="eW1s")
                    nc.sync.dma_start(eW1s_sb[:], eW1s_d[:, :])
                    eW2_sb = [encw.tile([P, D], bf16, tag=f"eW2_{k}", name=f"eW2_{k}")
                              for k in range(KD)]
                    for k in range(KD):
                        nc.sync.dma_start(eW2_sb[k][:], eW2_d[k * P:(k + 1) * P, :])
                    eb2_sb = encw.tile([1, D], bf16, tag="eb2")
                    nc.sync.dma_start(eb2_sb[:], eb2_d[:, :])
                    negc_sb = encw.tile([NUM_RBF, 1], f32, tag="negc")
                    nc.sync.dma_start(negc_sb[:], neg_centers_d[:, :])
                    iota2n_sb = encw.tile([P, 2 * NBLK], f32, tag="iota2n")
                    nc.sync.dma_start(iota2n_sb[:], iota2n_d[:, :])

                    # persistent rbfT tiles (one per slab index) with a ones row
                    rbfT_sb = [encrb.tile([NUM_RBF + 1, w], bf16, tag=f"rbfT{si}",
                                          name=f"rbfT{si}")
                               for si, (s0, w) in enumerate(slabs)]
                    for t, (s0, w) in zip(rbfT_sb, slabs):
                        # engines cannot address partition base 100; DMA can
                        nc.sync.dma_start(t[NUM_RBF:NUM_RBF + 1, :],
                                          ones_sq_b[:1, :w])

                    # stats PSUM: rows 0..N-1 = sum(e2), rows N..2N-1 = sum(e2^2)
                    stat_ps = [pcstat.tile([2 * NBLK, w], f32, tag=f"stat{si}",
                                           name=f"stat{si}")
                               for si, (s0, w) in enumerate(slabs)]

                    for b in range(NBLK):
                        dist_row = encsb.tile([1, EPB], bf16, tag="dist_row")
                        nc.sync.dma_start(dist_row[:], distT_d[b])
                        sel1 = encsb.tile([P, 2 * NBLK], bf16, tag="sel1")
                        nc.vector.tensor_scalar(out=sel1[:], in0=iota2n_sb[:],
                                                scalar1=float(b), scalar2=None,
                                                op0=OP.is_equal)
                        sel2 = encsb.tile([P, 2 * NBLK], bf16, tag="sel2")
                        nc.vector.tensor_scalar(out=sel2[:], in0=iota2n_sb[:],
                                                scalar1=float(NBLK + b), scalar2=None,
                                                op0=OP.is_equal)
                        for si, (s0, w) in enumerate(slabs):
                            # RBF: broadcast dist row, then (d-c)^2, exp(-g u)
                            rbf_ps = pcrbf.tile([NUM_RBF, SLAB], f32, tag="rbf_ps")
                            nc.tensor.matmul(rbf_ps[:, :w], ones_1r100[:, :],
                                             dist_row[:, s0:s0 + w],
                                             start=True, stop=True)
                            u_sb = encsb.tile([NUM_RBF, SLAB], f32, tag="u_sb")
                            nc.scalar.activation(u_sb[:, :w], rbf_ps[:, :w], AF.Square,
                                                 bias=negc_sb[:, :1])
                            nc.scalar.activation(rbfT_sb[si][:NUM_RBF, :w], u_sb[:, :w],
                                                 AF.Exp, scale=-float(gamma))
                            # e1 = silu(rbfT.T @ [eW1; eb1])  (transposed, per k)
                            e1s = []
                            for k in range(KD):
                                e1_ps = pce1.tile([P, SLAB], f32, tag="e1_ps")
                                nc.tensor.matmul(e1_ps[:, :w],
                                                 eW1s_sb[:, k * P:(k + 1) * P],
                                                 rbfT_sb[si][:, :w],
                                                 start=True, stop=True)
                                th = encsb.tile([P, SLAB], bf16, tag="th1")
                                nc.scalar.activation(th[:, :w], e1_ps[:, :w],
                                                     AF.Tanh, scale=0.5)
                                t = encsb.tile([P, SLAB], bf16, tag=f"e1s{k}",
                                               name=f"e1s{k}")
                                nc.vector.scalar_tensor_tensor(
                                    out=t[:, :w], in0=th[:, :w], scalar=1.0,
                                    in1=e1_ps[:, :w], op0=OP.add, op1=OP.mult)
                                e1s.append(t)
                            # e2 = e1 @ eW2 + eb2 (transposed, per out chunk m)
                            for m in range(KD):
                                e2_ps = pce2.tile([P, SLAB], f32, tag="e2_ps")
                                for k in range(KD):
                                    nc.tensor.matmul(e2_ps[:, :w],
                                                     eW2_sb[k][:, m * P:(m + 1) * P],
                                                     e1s[k][:, :w],
                                                     start=(k == 0), stop=False)
                                nc.tensor.matmul(e2_ps[:, :w],
                                                 eb2_sb[:, m * P:(m + 1) * P],
                                                 ones_sq_b[:1, :w],
                                                 start=False, stop=True)
                                e2s = encsb.tile([P, SLAB], bf16, tag="e2s")
                                nc.scalar.copy(e2s[:, :w], e2_ps[:, :w])
                                sq = encsb.tile([P, SLAB], bf16, tag="sqs")
                                nc.vector.tensor_tensor(out=sq[:, :w], in0=e2s[:, :w],
                                                        in1=e2s[:, :w], op=OP.mult)
                                acc0 = b == 0 and m == 0
                                nc.tensor.matmul(stat_ps[si][:, :w], sel1[:], e2s[:, :w],
                                                 start=acc0, stop=False)
                                last = b == NBLK - 1 and m == KD - 1
                                nc.tensor.matmul(stat_ps[si][:, :w], sel2[:], sq[:, :w],
                                                 start=False, stop=last)
                                nc.sync.dma_start(e2raw_d[b, m, :, s0:s0 + w],
                                                  e2s[:, :w])

                    # ---- pass 2: batched LayerNorm stats for all blocks ----
                    # round-trip through DRAM so mu/r2 halves can be re-read at
                    # partition base 0 (engines need aligned operand bases)
                    statsA = encw.tile([2 * NBLK, EPB], f32, tag="statsA")
                    for si, (s0, w) in enumerate(slabs):
                        nc.vector.tensor_copy(statsA[:, s0:s0 + w], stat_ps[si][:, :w])
                    nc.sync.dma_start(stats_d[:, :], statsA[:])
                    muD = encw.tile([NBLK, EPB], f32, tag="muD")
                    nc.sync.dma_start(muD[:], stats_d[:NBLK, :])
                    r2D = encw.tile([NBLK, EPB], f32, tag="r2D")
                    nc.sync.dma_start(r2D[:], stats_d[NBLK:, :])
                    nc.vector.tensor_scalar(out=muD[:], in0=muD[:], scalar1=1.0 / D,
                                            scalar2=None, op0=OP.mult)
                    mu2 = encw.tile([NBLK, EPB], f32, tag="mu2")
                    nc.scalar.activation(mu2[:], muD[:], AF.Square)
                    var = encw.tile([NBLK, EPB], f32, tag="var")
                    nc.vector.scalar_tensor_tensor(out=var[:], in0=r2D[:],
                                                   scalar=1.0 / D, in1=mu2[:],
                                                   op0=OP.mult, op1=OP.subtract)
                    nc.vector.tensor_scalar(out=var[:], in0=var[:], scalar1=0.0,
                                            scalar2=None, op0=OP.max)
                    nc.scalar.activation(var[:], var[:], AF.Ln, bias=eps_col[:NBLK, :1])
                    nc.scalar.activation(var[:], var[:], AF.Exp, scale=-0.5)  # rstd
                    nmu = encw.tile([NBLK, EPB], f32, tag="nmu")
                    nc.vector.scalar_tensor_tensor(out=nmu[:], in0=muD[:], scalar=-1.0,
                                                   in1=var[:], op0=OP.mult, op1=OP.mult)
                    nc.vector.tensor_copy(rstd16[:], var[:])
                    nc.vector.tensor_copy(nmu16[:], nmu[:])

                # ---- pass 3: enc = e2*outer(g,rstd) + outer(g,nmu) + beta ----
                with (
                    tc.tile_pool(name="enc3", bufs=2) as enc3,
                    tc.tile_pool(name="pc3a", bufs=3, space="PSUM") as pc3a,
                    tc.tile_pool(name="pc3b", bufs=3, space="PSUM") as pc3b,
                ):
                    for b in range(NBLK):
                        e2t = [enc3.tile([P, EPB], bf16, tag=f"e2t{k}",
                                         name=f"e2t{k}") for k in range(KD)]
                        wout = [enc3.tile([P, EPB], bf16, tag=f"wout{k}",
                                          name=f"wout{k}") for k in range(KD)]
                        for k in range(KD):
                            nc.sync.dma_start(e2t[k][:], e2raw_d[b, k])
                        # per-block rows at partition base 0 (SBUF->SBUF DMA)
                        rstd_row = enc3.tile([1, EPB], bf16, tag="rstd_row")
                        nc.sync.dma_start(rstd_row[:], rstd16[b:b + 1, :])
                        nmu_row = enc3.tile([1, EPB], bf16, tag="nmu_row")
                        nc.sync.dma_start(nmu_row[:], nmu16[b:b + 1, :])
                        for s0, w in slabs:
                            # rstd/nmu broadcast across partitions, shared by
                            # all 3 feature chunks; g/beta applied per chunk
                            # as per-partition scalars
                            pA = pc3a.tile([P, SLAB], f32, tag="pA")
                            nc.tensor.matmul(pA[:, :w], ones_row_b[:, :],
                                             rstd_row[:, s0:s0 + w],
                                             start=True, stop=True)
                            pB = pc3b.tile([P, SLAB], f32, tag="pB")
                            nc.tensor.matmul(pB[:, :w], ones_row_b[:, :],
                                             nmu_row[:, s0:s0 + w],
                                             start=True, stop=True)
                            pA16 = enc3.tile([P, SLAB], bf16, tag="pA16")
                            nc.scalar.copy(pA16[:, :w], pA[:, :w])
                            pB16 = enc3.tile([P, SLAB], bf16, tag="pB16")
                            nc.scalar.copy(pB16[:, :w], pB[:, :w])
                            for m in range(KD):
                                t1 = enc3.tile([P, SLAB], bf16, tag="t1")
                                nc.vector.tensor_tensor(out=t1[:, :w],
                                                        in0=e2t[m][:, s0:s0 + w],
                                                        in1=pA16[:, :w], op=OP.mult)
                                nc.vector.tensor_tensor(out=t1[:, :w],
                                                        in0=t1[:, :w],
                                                        in1=pB16[:, :w], op=OP.add)
                                nc.vector.tensor_scalar(
                                    out=wout[m][:, s0:s0 + w], in0=t1[:, :w],
                                    scalar1=eg_sb[:, m:m + 1],
                                    scalar2=ebeta_sb[:, m:m + 1],
                                    op0=OP.mult, op1=OP.add)
                        for k in range(KD):
                            nc.sync.dma_start(encT_d[b, k], wout[k][:])

            # =========================================================
            # Main layers
            # =========================================================
            with (
                tc.tile_pool(name="xrpool", bufs=1) as xrpool,
                tc.tile_pool(name="htpool", bufs=1) as htpool,
                tc.tile_pool(name="lw", bufs=2) as lw,
                tc.tile_pool(name="lep", bufs=1) as lep,
                tc.tile_pool(name="lsb", bufs=2) as lsb,
                tc.tile_pool(name="gat", bufs=2) as gat,
                tc.tile_pool(name="eetp", bufs=2) as eetp,
                tc.tile_pool(name="lps", bufs=3, space="PSUM") as lps,
                tc.tile_pool(name="lpt", bufs=2, space="PSUM") as lpt,
                tc.tile_pool(name="lpo", bufs=2, space="PSUM") as lpo,
            ):
                xr_sb = [xrpool.tile([P, D], bf16, tag=f"xr{b}", name=f"xr{b}")
                         for b in range(NBLK)]
                hT_sb = [[htpool.tile([P, P], bf16, tag=f"hT{b}_{k}",
                                      name=f"hT{b}_{k}") for k in range(KD)]
                         for b in range(NBLK)]
                def load_d1_weights(layer):
                    Wl_sb = [lw.tile([P, D], bf16, tag=f"Wl{k}", name=f"Wl{k}")
                             for k in range(KD)]
                    for k in range(KD):
                        nc.sync.dma_start(Wl_sb[k][:], Wl_d[layer, k * P:(k + 1) * P, :])
                    bl_sb = lw.tile([1, D], bf16, tag="bl")
                    nc.sync.dma_start(bl_sb[:], bl_d[layer])
                    return Wl_sb, bl_sb

                def emit_d1(blocks, Wl_sb, bl_sb):
                    # hT + xl for the given blocks (xl feeds the AllGather)
                    for b in blocks:
                        for k in range(KD):
                            pt = lpt.tile([P, P], f32, tag="pt")
                            nc.tensor.transpose(pt[:], h_sb[b][:, k * P:(k + 1) * P], ident[:])
                            nc.vector.tensor_copy(hT_sb[b][k][:], pt[:])
                        pxl = lps.tile([P, D], f32, tag="ps")
                        for k in range(KD):
                            nc.tensor.matmul(pxl[:], hT_sb[b][k][:], Wl_sb[k][:],
                                             start=(k == 0), stop=False)
                        nc.tensor.matmul(pxl[:], ones_row_b[:, :P], bl_sb[:],
                                         start=False, stop=True)
                        xl_t = lsb.tile([P, D], bf16, tag="xl_t")
                        nc.vector.tensor_copy(xl_t[:], pxl[:])
                        nc.sync.dma_start(xl_shard_d[b * P:(b + 1) * P, :], xl_t[:])

                # layer 0's xl comes straight from the stage-B embeddings
                Wn = load_d1_weights(0)
                emit_d1(range(NBLK), *Wn)

                for layer in range(L):
                    # ---- layer weights (xr / edge / epilogue) ----
                    Wr_sb = [lw.tile([P, D], bf16, tag=f"Wr{k}", name=f"Wr{k}")
                             for k in range(KD)]
                    We_sb = [lw.tile([P, D], bf16, tag=f"We{k}", name=f"We{k}")
                             for k in range(KD)]
                    for k in range(KD):
                        nc.sync.dma_start(Wr_sb[k][:], Wr_d[layer, k * P:(k + 1) * P, :])
                        nc.sync.dma_start(We_sb[k][:], We_d[layer, k * P:(k + 1) * P, :])
                    br_sb = lw.tile([1, D], bf16, tag="br")
                    nc.sync.dma_start(br_sb[:], br_d[layer])
                    attb_sb = lw.tile([P, D], bf16, tag="attb")
                    nc.sync.dma_start(attb_sb[:], att_b_d[layer])
                    bnsc_sb = lw.tile([P, D], bf16, tag="bnsc")
                    nc.sync.dma_start(bnsc_sb[:], bnsc_b_d[layer])
                    bnsh_sb = lw.tile([P, D], bf16, tag="bnsh")
                    nc.sync.dma_start(bnsh_sb[:], bnsh_b_d[layer])

                    # ---- AllGather xl (xr compute overlaps with it) ----
                    nc.gpsimd.collective_compute(
                        "AllGather", OP.bypass, replica_groups=rg,
                        ins=[xl_shard_d[:, :]], outs=[xl_full_d[:, :]],
                    )

                    # ---- stage D part 2: xr for every block ----
                    for b in range(NBLK):
                        pxr = lps.tile([P, D], f32, tag="ps")
                        for k in range(KD):
                            nc.tensor.matmul(pxr[:], hT_sb[b][k][:], Wr_sb[k][:],
                                             start=(k == 0), stop=False)
                        nc.tensor.matmul(pxr[:], ones_row_b[:, :P], br_sb[:],
                                         start=False, stop=True)
                        nc.vector.tensor_copy(xr_sb[b][:], pxr[:])

                    if layer < L - 1:
                        Wn = load_d1_weights(layer + 1)

                    def emit_epilogue_group(a, b_end):
                        # softmax-normalize + bn + silu + residual for blocks
                        # [a, b_end); contiguous bf16 tiles keep DVE at 2x rate
                        n = b_end - a
                        nc.vector.tensor_scalar(out=den_f[:, a:b_end, :],
                                                in0=den_all[:, a:b_end, :],
                                                scalar1=1e-16, scalar2=None,
                                                op0=OP.add)
                        nc.vector.reciprocal_approx_fast(rec_f[:, a:b_end, :],
                                                         den_f[:, a:b_end, :])
                        nc.vector.tensor_copy(rec16[:, a:b_end, :],
                                              rec_f[:, a:b_end, :])
                        nv4 = num_all[:, a:b_end, :].rearrange(
                            "p n (h c) -> p n h c", h=H)
                        rec_v = rec16[:, a:b_end, :].rearrange(
                            "p n (h o) -> p n h o", o=1).to_broadcast([P, n, H, C])
                        nc.vector.tensor_tensor(out=nv4, in0=nv4, in1=rec_v,
                                                op=OP.mult)
                        nv = num_all[:, a:b_end, :]
                        bnsc_v = bnsc_sb[:].rearrange("p (o d) -> p o d", o=1) \
                            .to_broadcast([P, n, D])
                        bnsh_v = bnsh_sb[:].rearrange("p (o d) -> p o d", o=1) \
                            .to_broadcast([P, n, D])
                        nc.vector.tensor_tensor(out=nv, in0=nv, in1=bnsc_v, op=OP.mult)
                        nc.vector.tensor_tensor(out=nv, in0=nv, in1=bnsh_v, op=OP.add)
                        nc.scalar.activation(th_all[:, a:b_end, :], nv, AF.Tanh)
                        nc.vector.scalar_tensor_tensor(out=nv,
                                                       in0=th_all[:, a:b_end, :],
                                                       scalar=1.0, in1=nv,
                                                       op0=OP.add, op1=OP.mult)
                        for b in range(a, b_end):
                            nc.vector.tensor_tensor(out=h_sb[b][:], in0=h_sb[b][:],
                                                    in1=num_all[:, b, :], op=OP.add)

                    # ---- stage E: edge message passing ----
                    num_all = lep.tile([P, NBLK, D], bf16, tag="num_all")
                    den_all = lep.tile([P, NBLK, H], bf16, tag="den_all")
                    den_f = lep.tile([P, NBLK, H], f32, tag="den_f")
                    rec_f = lep.tile([P, NBLK, H], f32, tag="rec_f")
                    rec16 = lep.tile([P, NBLK, H], bf16, tag="rec16")
                    th_all = lep.tile([P, NBLK, D], bf16, tag="th_all")
                    GRP = 5
                    for b in range(NBLK):
                        gix = gat.tile([P, EPB // 16], i16, tag="gix")
                        nc.sync.dma_start(gix[:], gidx_d[b])
                        ohs_t = gat.tile([P, CPB, P], bf16, tag="ohs_t")
                        nc.sync.dma_start(ohs_t[:], ohs_d[b])
                        ohg_t = gat.tile([P, CPB, P], bf16, tag="ohg_t")
                        nc.sync.dma_start(ohg_t[:], ohg_d[b])
                        eet = [eetp.tile([P, EPB], bf16, tag=f"eet{k}", name=f"eet{k}")
                               for k in range(KD)]
                        for k in range(KD):
                            nc.sync.dma_start(eet[k][:], encT_d[b, k])
                        xsg = eetp.tile([P, CPB, D], bf16, tag="xsg")
                        nc.gpsimd.dma_gather(xsg[:], xl_full_d[:, :], gix[:], EPB, EPB, D,
                                             single_packet=False)
                        psum_o = lpo.tile([P, D + H], f32, tag="po")
                        for c in range(CPB):
                            xsrc = xsg[:, c]
                            psum_s = lps.tile([P, D], f32, tag="ps")
                            for k in range(KD):
                                nc.tensor.matmul(psum_s[:], eet[k][:, c * P:(c + 1) * P],
                                                 We_sb[k][:], start=(k == 0), stop=False)
                            nc.tensor.matmul(psum_s[:], ohg_t[:, c, :], xr_sb[b][:],
                                             start=False, stop=True)
                            s_sb = lsb.tile([P, D], bf16, tag="s_sb")
                            nc.vector.tensor_tensor(out=s_sb[:], in0=psum_s[:],
                                                    in1=xsrc, op=OP.add)
                            m_sb = lsb.tile([P, D], bf16, tag="m_sb")
                            if HW_ACTS:
                                nc.scalar.activation(m_sb[:], s_sb[:], AF.Prelu, alpha=0.2)
                            else:
                                nc.scalar.activation(m_sb[:], s_sb[:], AF.Relu)
                            t_sb = lsb.tile([P, D], bf16, tag="t_sb")
                            nc.vector.tensor_tensor(out=t_sb[:], in0=m_sb[:], in1=attb_sb[:],
                                                    op=OP.mult)
                            lg = lsb.tile([P, H], f32, tag="lg")
                            nc.vector.tensor_reduce(
                                out=lg[:], in_=t_sb[:].rearrange("p (h c) -> p h c", h=H),
                                axis=mybir.AxisListType.X, op=OP.add)
                            z_sb = lsb.tile([P, D + H], bf16, tag="z_sb")
                            nc.scalar.activation(z_sb[:, D:], lg[:], AF.Exp)
                            el_b = z_sb[:, D:].rearrange("p (h o) -> p h o", o=1).to_broadcast([P, H, C])
                            nc.vector.tensor_tensor(
                                out=z_sb[:, :D].rearrange("p (h c) -> p h c", h=H),
                                in0=xsrc.rearrange("p (h c) -> p h c", h=H),
                                in1=el_b, op=OP.mult)
                            nc.tensor.matmul(psum_o[:], ohs_t[:, c, :], z_sb[:],
                                             start=(c == 0), stop=(c == CPB - 1))
                        # stash numerators+denominators; epilogue is grouped
                        nc.scalar.copy(num_all[:, b, :], psum_o[:, :D])
                        nc.scalar.copy(den_all[:, b, :], psum_o[:, D:])
                        if b % GRP == GRP - 1:
                            a = b - (GRP - 1)
                            emit_epilogue_group(a, b + 1)
                            if layer < L - 1:
                                emit_d1(range(a, b + 1), *Wn)

            # =========================================================
            # Stage F: pooling + head
            # =========================================================
            with (
                tc.tile_pool(name="fsb", bufs=3) as fsb,
                tc.tile_pool(name="fkeep", bufs=1) as fkeep,
                tc.tile_pool(name="fps", bufs=2, space="PSUM") as fps,
                tc.tile_pool(name="fsum", bufs=1, space="PSUM") as fsum,
            ):
                psum_sum = fsum.tile([G, D], f32, tag="psum_sum")
                psum_cnt = fsum.tile([1, G], f32, tag="psum_cnt")
                bm = [fkeep.tile([P, MAXG * NBLK], f32, tag=f"bm{k}", name=f"bm{k}")
                      for k in range(KD)]
                for b in range(NBLK):
                    ohb = fsb.tile([P, G], f32, tag="ohb")
                    nc.sync.dma_start(ohb[:], oh_d[b])
                    mab = fsb.tile([P, MAXG], f32, tag="mab")
                    nc.sync.dma_start(mab[:], maskAB_d[b])
                    nc.tensor.matmul(psum_sum[:], ohb[:], h_sb[b][:],
                                     start=(b == 0), stop=(b == NBLK - 1))
                    nc.tensor.matmul(psum_cnt[:], ones_col[:, :1], ohb[:],
                                     start=(b == 0), stop=(b == NBLK - 1))
                    for half in range(MAXG):
                        mh = fsb.tile([P, D], f32, tag="mh")
                        nc.vector.tensor_scalar(out=mh[:], in0=h_sb[b][:],
                                                scalar1=mab[:, half:half + 1], scalar2=None,
                                                op0=OP.add)
                        for k in range(KD):
                            pt = fps.tile([P, P], f32, tag="pt")
                            nc.tensor.transpose(pt[:], mh[:, k * P:(k + 1) * P], ident[:])
                            mt = fsb.tile([P, P], f32, tag="mt")
                            nc.vector.tensor_copy(mt[:], pt[:])
                            nc.vector.tensor_reduce(
                                out=bm[k][:, MAXG * b + half:MAXG * b + half + 1],
                                in_=mt[:], axis=mybir.AxisListType.X, op=OP.max)
                # combine per-graph maxes
                gmaxT = [fkeep.tile([P, G], f32, tag=f"gmaxT{k}", name=f"gmaxT{k}")
                         for k in range(KD)]
                for g in range(G):
                    cr = fsb.tile([1, MAXG * NBLK], f32, tag="cr")
                    nc.sync.dma_start(cr[:], cmb_d[g])
                    pc = fps.tile([P, MAXG * NBLK], f32, tag="pt")
                    nc.tensor.matmul(pc[:], ones_row[:, :P], cr[:], start=True, stop=True)
                    for k in range(KD):
                        mm = fsb.tile([P, MAXG * NBLK], f32, tag="mm")
                        nc.vector.tensor_tensor(out=mm[:], in0=bm[k][:], in1=pc[:], op=OP.add)
                        nc.vector.tensor_reduce(out=gmaxT[k][:, g:g + 1], in_=mm[:],
                                                axis=mybir.AxisListType.X, op=OP.max)
                # partial sums to DRAM
                sum_sb = fsb.tile([G, D], f32, tag="sum_sb")
                nc.vector.tensor_copy(sum_sb[:], psum_sum[:])
                for k in range(KD):
                    pt = fps.tile([P, G], f32, tag="pt")
                    nc.tensor.transpose(pt[:, :G], sum_sb[:, k * P:(k + 1) * P], ident[:G, :G])
                    st = fsb.tile([P, G], f32, tag="st")
                    nc.vector.tensor_copy(st[:], pt[:, :G])
                    nc.sync.dma_start(pool_part_d[k * P:(k + 1) * P, :], st[:])
                    nc.sync.dma_start(pool_part_d[D + k * P:D + (k + 1) * P, :], gmaxT[k][:])
                cntT = fsb.tile([1, G], f32, tag="cntT")
                nc.vector.tensor_copy(cntT[:], psum_cnt[:])
                nc.sync.dma_start(pool_part_d[2 * D:2 * D + 1, :], cntT[:])

                # ---- tiny AllGather of partials ----
                nc.gpsimd.collective_compute(
                    "AllGather", OP.bypass, replica_groups=rg,
                    ins=[pool_part_d[:, :]], outs=[pool_all_d[:, :]],
                )

                # ---- combine + head (replicated on all devices) ----
                n_dev_ = n_dev
                STRIDE = 2 * D + 1
                meanT = [fkeep.tile([P, G], f32, tag=f"meanT{k}", name=f"meanT{k}")
                         for k in range(KD)]
                maxT = [fkeep.tile([P, G], f32, tag=f"maxT{k}", name=f"maxT{k}")
                        for k in range(KD)]
                cnt_tot = fkeep.tile([1, G], f32, tag="cnt_tot")
                for dv in range(n_dev_):
                    base = dv * STRIDE
                    for k in range(KD):
                        ts = fsb.tile([P, G], f32, tag="ts")
                        nc.sync.dma_start(ts[:], pool_all_d[base + k * P:base + (k + 1) * P, :])
                        tm = fsb.tile([P, G], f32, tag="tm")
                        nc.sync.dma_start(tm[:], pool_all_d[base + D + k * P:base + D + (k + 1) * P, :])
                        if dv == 0:
                            nc.vector.tensor_copy(meanT[k][:], ts[:])
                            nc.vector.tensor_copy(maxT[k][:], tm[:])
                        else:
                            nc.vector.tensor_tensor(out=meanT[k][:], in0=meanT[k][:],
                                                    in1=ts[:], op=OP.add)
                            nc.vector.tensor_tensor(out=maxT[k][:], in0=maxT[k][:],
                                                    in1=tm[:], op=OP.max)
                    tc_ = fsb.tile([1, G], f32, tag="tc_")
                    nc.sync.dma_start(tc_[:], pool_all_d[base + 2 * D:base + 2 * D + 1, :])
                    if dv == 0:
                        nc.vector.tensor_copy(cnt_tot[:], tc_[:])
                    else:
                        nc.vector.tensor_tensor(out=cnt_tot[:], in0=cnt_tot[:], in1=tc_[:],
                                                op=OP.add)
                nc.vector.tensor_scalar(out=cnt_tot[:], in0=cnt_tot[:], scalar1=1.0,
                                        scalar2=None, op0=OP.max)
                inv_cnt = fkeep.tile([1, G], f32, tag="inv_cnt")
                nc.vector.reciprocal(inv_cnt[:], cnt_tot[:])
                pic = fps.tile([P, G], f32, tag="pt")
                nc.tensor.matmul(pic[:], ones_row[:, :P], inv_cnt[:], start=True, stop=True)
                for k in range(KD):
                    nc.vector.tensor_tensor(out=meanT[k][:], in0=meanT[k][:], in1=pic[:],
                                            op=OP.mult)
                hgT = meanT + maxT          # 6 k-tiles of [128, G] = hg transposed

                # head weights
                pW_sb = [fkeep.tile([P, D], f32, tag=f"pW{k}", name=f"pW{k}")
                         for k in range(2 * KD)]
                for k in range(2 * KD):
                    nc.sync.dma_start(pW_sb[k][:], pW_d[k * P:(k + 1) * P, :])
                pb_sb = fkeep.tile([1, D], f32, tag="pb")
                nc.sync.dma_start(pb_sb[:], pb_d[:, :])
                hW1_sb = [fkeep.tile([P, D], f32, tag=f"hW1_{k}", name=f"hW1_{k}")
                          for k in range(KD)]
                for k in range(KD):
                    nc.sync.dma_start(hW1_sb[k][:], hW1_d[k * P:(k + 1) * P, :])
                hb1_sb = fkeep.tile([1, D], f32, tag="hb1")
                nc.sync.dma_start(hb1_sb[:], hb1_d[:, :])
                hW2_sb = [fkeep.tile([P, D // 2], f32, tag=f"hW2_{k}", name=f"hW2_{k}")
                          for k in range(KD)]
                for k in range(KD):
                    nc.sync.dma_start(hW2_sb[k][:], hW2_d[k * P:(k + 1) * P, :])
                hb2_sb = fkeep.tile([1, D // 2], f32, tag="hb2")
                nc.sync.dma_start(hb2_sb[:], hb2_d[:, :])
                hW3_sb = fkeep.tile([P, 2], f32, tag="hW3")
                nc.sync.dma_start(hW3_sb[:], hW3_d[:, :].rearrange("(k p) o -> p (k o)", p=P))
                hb3_sb = fkeep.tile([1, 1], f32, tag="hb3")
                nc.sync.dma_start(hb3_sb[:], hb3_d[:, :])

                def mlp_layer(in_tiles, W_tiles, b_row, out_feats, lid, act=True):
                    outs = []
                    n_out_tiles = (out_feats + P - 1) // P
                    for m in range(n_out_tiles):
                        mw = min(P, out_feats - m * P)
                        pm = fps.tile([P, G], f32, tag="ph", name=f"ph{lid}_{m}")
                        for k, (it, wt) in enumerate(zip(in_tiles, W_tiles)):
                            nc.tensor.matmul(pm[:mw, :], wt[:, m * P:m * P + mw], it[:],
                                             start=(k == 0), stop=False)
                        nc.tensor.matmul(pm[:mw, :], b_row[:, m * P:m * P + mw],
                                         ones_row[:, :G], start=False, stop=True)
                        ot = fkeep.tile([P, G], f32, tag=f"ot{lid}_{m}", name=f"ot{lid}_{m}")
                        if mw < P:
                            nc.vector.memset(ot[mw:, :], 0.0)
                        if act:
                            emit_silu(fsb, ot[:mw, :], pm[:mw, :], [mw, G])
                        else:
                            nc.vector.tensor_copy(ot[:mw, :], pm[:mw, :])
                        outs.append(ot)
                    return outs

                h1 = mlp_layer(hgT, pW_sb, pb_sb, D, 1)
                h2 = mlp_layer(h1, hW1_sb, hb1_sb, D, 2)
                h3 = mlp_layer(h2, hW2_sb, hb2_sb, D // 2, 3)
                pf = fps.tile([1, G], f32, tag="pf")
                nc.tensor.matmul(pf[:], hW3_sb[:, 0:1], h3[0][:], start=True, stop=False)
                nc.tensor.matmul(pf[:], hW3_sb[:, 1:2], h3[1][:], start=False, stop=False)
                nc.tensor.matmul(pf[:], hb3_sb[:, :1], ones_row[:, :G], start=False, stop=True)
                fo = fsb.tile([1, G], f32, tag="fo")
                nc.vector.tensor_copy(fo[:], pf[:])
                nc.sync.dma_start(out_d[:].rearrange("(o g) -> o g", o=1), fo[:])

    nc.compile()
    return nc


# --------------------------------------------------------------------------
# entry point
# --------------------------------------------------------------------------

def kernel(**inputs):
    n_dev = 8
    meta, rep, devs = prep_host(inputs, n_dev)
    nc = build_program(meta)

    in_maps = []
    for d in range(n_dev):
        m = dict(rep)
        m.update(devs[d])
        in_maps.append(m)

    global LAST_RESULTS
    res = run_bass_kernel_spmd(nc, in_maps, core_ids=list(range(n_dev)),
                               trace=TRACE)
    LAST_RESULTS = res
    out = np.asarray(res.results[0]["out"], np.float32)
    return out
